# revision 14
# baseline (speedup 1.0000x reference)
"""AttnBlock (GroupNorm -> qkv 1x1 -> NxN spatial attention -> proj -> residual)
for Trainium2, SPMD over 8 NeuronCores.

Sharding: core = (batch b in 0..3, query-half qh in 0..1): the host rotates
the spatial columns so each core's 2048 queries are columns 0..2047 of its
input; keys are the full 4096 columns (key order is irrelevant to softmax
attention, and only query columns are written back).

Algebraic restructure vs the straightforward q/k/v/proj pipeline (exact, no
approximation):
  * scores_ij = (Wq hn_i + bq).(Wk hn_j + bk); per-query terms cancel in the
    j-softmax, so scores ~ P_i . hn_j with P = scale*((Wk^T Wq) hn + Wk^T bq).
    The K projection disappears: keys are raw hn, queries get one projection
    by the host-precomputed 512x512 matrix Wk^T Wq.
  * attention rows sum to 1, so the output projection commutes through the
    value sum: out_i = sum_j a_ij (Wp Wv hn)_j + (Wp bv + bp).  V is projected
    once by the host-precomputed Wp Wv; the separate proj matmul disappears.
Both precomputed matrices are weight-only (input-independent), like the bias
folds.  This removes ~25% of the tensor-engine work.

Engine budget per 512-query attention block (PE is the roofline at ~27.5us):
PE S^T 13.7 + AV 13.7; ACT exp 17.8 + psum evacuation 2.7; DVE denominator
tree ~8 + reciprocal 3.4; Pool normalize/bias/residual chain ~16 + out DMA.
The AV psum is evacuated by plain ACT copies so no PSUM buffer ever waits on
the softmax denominator (which needs the full tree + reciprocal); the
denominator is reduced in two halves with the ones-matmul accumulating both
into one PSUM bank, so only half a tree remains after the last exp.

GroupNorm: x is loaded once as fp8 (stats and normalize both read it; the
fp32 residual copy streams in during the attention phase).  Chunk 0 uses
classic sum/sumsq on ACT, chunks 1-3 single-pass BN_STATS on DVE, per-chunk
group chains run interleaved with the later chunks' stats, normalizes split
ACT/DVE.  Engines execute in program order, so ops are emitted in data-arrival
order; the x DMAs are first on both trigger queues.
"""

import numpy as np

_B, _C, _HW = 4, 512, 64 * 64  # batch, channels, spatial N
_N = _HW                       # 4096
_NQ = _N // 2                  # queries per core
_G = 32                        # groupnorm groups
_EPS = 1e-6
_NCORES = 8
_CCH = _C // 128               # 4 channel chunks

# fp8 range scaling (host pre-mult, device descale; all powers of two)
_SA = 4096.0   # on Wk^T Wq entries (~4e-4 std)
_SP = 256.0    # on P in SBUF (~0.01 std)
_SV = 512.0    # on Wp Wv entries (~0.009 std)
_SV2 = 16.0    # on V' in SBUF (~0.2 std); folded back via the ones-matrix

_cached = None  # (nc,) built Bass program, reused across kernel() calls


def _legalize_single_wait(nc, mybir):
    """This container's walrus codegen accepts at most ONE sync-wait per
    instruction. Tile emits N-wait instructions; hoist the extras onto
    injected same-engine NOPs placed immediately before."""
    ctr = 0
    for f in nc.m.functions:
        for bb in f.blocks:
            out = []
            changed = False
            for inst in bb.instructions:
                si = inst.sync_info
                if si is not None and len(si.on_wait) > 1:
                    waits = list(si.on_wait)
                    for w in waits[:-1]:
                        ctr += 1
                        out.append(mybir.InstNoOp(
                            name=f"I-legalize-wait-{ctr}",
                            engine=inst.engine,
                            sync_info=mybir.SyncInfo(on_wait=[w], on_update=[]),
                        ))
                    inst.sync_info = mybir.SyncInfo(
                        on_wait=[waits[-1]], on_update=list(si.on_update))
                    changed = True
                out.append(inst)
            if changed:
                bb.instructions = out


def _build_program():
    import concourse.bass as bass
    import concourse.tile as tile
    import concourse.mybir as mybir

    f32 = mybir.dt.float32
    bf16 = mybir.dt.bfloat16
    fp8 = mybir.dt.float8e4
    DR = mybir.MatmulPerfMode.DoubleRow
    AF = mybir.ActivationFunctionType
    OP = mybir.AluOpType

    nc = bass.Bass(name="attnblock")

    xb8 = nc.declare_dram_parameter("xb8", [_C, _N], fp8, isOutput=False)
    xqf = nc.declare_dram_parameter("xqf", [_C, _NQ], f32, isOutput=False)
    wPT = nc.declare_dram_parameter("wPT", [128, _CCH * _C], fp8, isOutput=False)
    wVT = nc.declare_dram_parameter("wVT", [128, _CCH * _C], fp8, isOutput=False)
    # all small [128, x] f32 constants packed into one tensor:
    # [bPa(4) | bPb(4) | bpe2(4) | gnw2(4) | gnb2(4) | gmat(8)]
    consts = nc.declare_dram_parameter("consts", [128, 28], f32, isOutput=False)
    obc16 = nc.declare_dram_parameter("obc16", [128, 128], bf16, isOutput=False)
    gexp = nc.declare_dram_parameter("gexp", [8, 128], f32, isOutput=False)
    out_d = nc.declare_dram_parameter("out", [_C, _NQ], f32, isOutput=True)

    QW = _N // 4    # 1024: classic-stat quarter
    HW2 = _N // 2

    with tile.TileContext(nc) as tc:
        with (
            tc.tile_pool(name="singles", bufs=1) as singles,
            tc.tile_pool(name="persist", bufs=1) as persist,
        ):
            # ---- input DMAs first on both trigger queues -----------------
            # x (fp8) chunks, halves spread across the two DMA-trigger
            # engines; halves keep the per-partition line at 2KB
            dma_engs = [nc.sync, nc.gpsimd]
            xts = []
            k = 0
            for ci in range(_CCH):
                xt = persist.tile([128, _N], fp8, tag=f"xt{ci}",
                                  name=f"xt{ci}")
                for h in range(2):
                    eng = dma_engs[k % 2]
                    k += 1
                    sl = slice(h * HW2, (h + 1) * HW2)
                    eng.dma_start(out=xt[:, sl],
                                  in_=xb8[ci * 128:(ci + 1) * 128, sl])
                xts.append(xt)

            # ---- constants / weights -------------------------------------
            sb_consts = singles.tile([128, 28], f32, tag="consts")
            nc.sync.dma_start(out=sb_consts, in_=consts[:, :])
            sb_bPa = sb_consts[:, 0:4]    # S_A*scale*(Wk^T bq)   (DVE copies)
            sb_bPb = sb_consts[:, 4:8]    # S_P*scale*(Wk^T bq)   (ACT copies)
            sb_bpe = sb_consts[:, 8:12]   # Wp bv + bp
            sb_gnw = sb_consts[:, 12:16]
            sb_gnb = sb_consts[:, 16:20]
            sb_gmat = sb_consts[:, 20:28]  # eye-repeat(8)/16
            sb_gexp = singles.tile([8, 128], f32, tag="gexp")
            nc.sync.dma_start(out=sb_gexp, in_=gexp[:, :])
            sb_obc16 = singles.tile([128, 128], bf16, tag="obc16")
            nc.sync.dma_start(out=sb_obc16, in_=obc16[:, :])
            # the two precombined weight matrices (needed ~30us in)
            w_tiles = {}
            for nm, src, eng in (("wP", wPT, nc.sync),
                                 ("wV", wVT, nc.gpsimd)):
                t = singles.tile([128, _CCH, _C], fp8, tag=f"w_{nm}",
                                 name=f"w_{nm}")
                eng.dma_start(
                    out=t, in_=src.rearrange("p (a f) -> p a f", a=_CCH))
                w_tiles[nm] = t

            sb_eps8 = singles.tile([8, 1], f32, tag="eps8")
            nc.vector.memset(sb_eps8, _EPS)
            sb_warm = singles.tile([128, 1], f32, tag="warm1")
            nc.vector.memset(sb_warm, 1.0)
            # ACT table prep: GN needs Sqrt/Square/Identity, all served by
            # the sqrt_and_others table.  Exp is touched once after the last
            # GN ACT op so the exp table load lands during the projections.
            sb_actw = singles.tile([8, 2], f32, tag="actw")
            nc.scalar.activation(out=sb_actw[:, 0:1], in_=sb_eps8, func=AF.Sqrt)
            nc.scalar.activation(out=sb_actw[:, 1:2], in_=sb_eps8,
                                 func=AF.Square)

            # normalize constants per chunk: mu', rstd', -mu'*rstd'
            musig = singles.tile([128, _CCH, 3], f32, tag="musig")

            # hn (normalized x, fp8) packed [c_lo, chunk, N]
            hn_t = persist.tile([128, _CCH, _N], fp8, tag="hn")

            # ---- phase 1: GroupNorm --------------------------------------
            with (
                tc.tile_pool(name="gn_small", bufs=2) as gn_small,
                tc.tile_pool(name="gn_psum", bufs=2, space="PSUM") as gn_psum,
                tc.tile_pool(name="warm_psum", bufs=1, space="PSUM") as warm_psum,
            ):
                # PE warm-up: keep the tensor engine busy through the GN head
                # so the HAM clock is ramped when the projections start.
                warm_ps = warm_psum.tile([128, 512], f32, tag="warm")

                def warm(n_small, n_big):
                    for _ in range(n_small):
                        nc.tensor.matmul(warm_ps[0:1, 0:1], lhsT=sb_warm,
                                         rhs=sb_warm, start=True, stop=True)
                    for _ in range(n_big):
                        nc.tensor.matmul(warm_ps, lhsT=xts[0][:, 0:128],
                                         rhs=xts[0][:, 0:512],
                                         start=True, stop=True)

                warm(80, 12)

                # per chunk: stats -> group chain -> normalize, interleaved
                # so chunk c's chain runs while chunk c+1's stats stream in.
                # Chunk 0: classic sum/sumsq on ACT (ACT is otherwise idle
                # early); chunks 1-3: single-pass BN_STATS on DVE.
                for ci in range(_CCH):
                    xt = xts[ci]
                    t2 = gn_small.tile([128, 2], f32, tag=f"t2_{ci}")
                    if ci == 0:
                        spart = gn_small.tile([128, 4], f32, tag="spart")
                        qpart = gn_small.tile([128, 4], f32, tag="qpart")
                        for h in range(4):
                            qs = slice(h * QW, (h + 1) * QW)
                            nc.scalar.activation(out=hn_t[:, ci, qs],
                                                 in_=xt[:, qs],
                                                 func=AF.Square,
                                                 accum_out=qpart[:, h:h + 1])
                            nc.scalar.activation(out=hn_t[:, ci, qs],
                                                 in_=xt[:, qs],
                                                 func=AF.Identity,
                                                 accum_out=spart[:, h:h + 1])
                        nc.vector.reduce_sum(out=t2[:, 0:1], in_=spart,
                                             axis=mybir.AxisListType.XYZW)
                        nc.vector.reduce_sum(out=t2[:, 1:2], in_=qpart,
                                             axis=mybir.AxisListType.XYZW)
                        nc.vector.tensor_scalar_mul(
                            out=t2[:, 0:1], in0=t2[:, 0:1], scalar1=1.0 / _N)
                        nc.vector.tensor_scalar_mul(
                            out=t2[:, 1:2], in0=t2[:, 1:2], scalar1=1.0 / _N)
                    else:
                        bn8 = gn_small.tile([128, 8, 6], f32, tag=f"bn8_{ci}")
                        for s in range(8):
                            nc.vector.bn_stats(
                                out=bn8[:, s, :],
                                in_=xt[:, s * 512:(s + 1) * 512])
                        mv = gn_small.tile([128, 2], f32, tag=f"mv_{ci}")
                        nc.vector.bn_aggr(out=mv, in_=bn8)
                        sqm = gn_small.tile([128, 1], f32, tag="sqm")
                        nc.vector.tensor_mul(sqm, mv[:, 0:1], mv[:, 0:1])
                        nc.vector.tensor_copy(out=t2[:, 0:1], in_=mv[:, 0:1])
                        nc.vector.tensor_tensor(
                            out=t2[:, 1:2], in0=mv[:, 1:2], in1=sqm,
                            op=OP.add)
                    # group chain (PSUM copies on DVE so the ACT queue stays
                    # free for the normalizes)
                    pg = gn_psum.tile([8, 2], f32, tag="pg")
                    nc.tensor.matmul(pg, lhsT=sb_gmat, rhs=t2,
                                     start=True, stop=True)
                    gs = gn_small.tile([8, 2], f32, tag="gs")
                    nc.vector.tensor_copy(out=gs, in_=pg)
                    # var_g = m2 - mu^2 ; rstd_g = 1/sqrt(var+eps)
                    musq = gn_small.tile([8, 1], f32, tag="musq")
                    nc.vector.tensor_mul(musq, gs[:, 0:1], gs[:, 0:1])
                    nc.vector.tensor_tensor(
                        out=gs[:, 1:2], in0=gs[:, 1:2], in1=musq,
                        op=OP.subtract)
                    sq8 = gn_small.tile([8, 1], f32, tag="sq8")
                    nc.scalar.activation(
                        out=sq8, in_=gs[:, 1:2], func=AF.Sqrt, bias=sb_eps8)
                    nc.vector.reciprocal(out=gs[:, 1:2], in_=sq8)
                    # broadcast to channels: [128, 2] = gexp.T @ [mu_g, rstd_g]
                    pc = gn_psum.tile([128, 2], f32, tag="pc")
                    nc.tensor.matmul(pc, lhsT=sb_gexp, rhs=gs, start=True,
                                     stop=True)
                    pcs = gn_small.tile([128, 2], f32, tag="pcs")
                    nc.vector.tensor_copy(out=pcs, in_=pc)
                    # fold gamma/beta: rstd' = rstd*gamma; mu' = mu - beta/rstd'
                    nc.vector.tensor_mul(
                        musig[:, ci, 1:2], pcs[:, 1:2], sb_gnw[:, ci:ci + 1])
                    rec = gn_small.tile([128, 1], f32, tag="rec")
                    nc.vector.reciprocal(out=rec, in_=musig[:, ci, 1:2])
                    bs = gn_small.tile([128, 1], f32, tag="bs")
                    nc.vector.tensor_mul(bs, sb_gnb[:, ci:ci + 1], rec)
                    nc.vector.tensor_tensor(
                        out=musig[:, ci, 0:1], in0=pcs[:, 0:1], in1=bs,
                        op=OP.subtract)
                    nc.vector.scalar_tensor_tensor(
                        out=musig[:, ci, 2:3], in0=musig[:, ci, 0:1],
                        scalar=-1.0, in1=musig[:, ci, 1:2],
                        op0=OP.mult, op1=OP.mult)
                    # hn = (x - mu') * rstd' (fp8): chunks 0-2 on ACT (DVE is
                    # streaming later chunks' stats), chunk 3 on DVE (free
                    # right after its chain; DVE fp8 normalize is ~2x ACT's)
                    if ci < 3:
                        nc.scalar.activation(
                            out=hn_t[:, ci, :], in_=xt,
                            func=AF.Identity, scale=musig[:, ci, 1:2],
                            bias=musig[:, ci, 2:3])
                    else:
                        nc.vector.tensor_scalar(
                            out=hn_t[:, ci, :], in0=xt,
                            scalar1=musig[:, ci, 0:1],
                            scalar2=musig[:, ci, 1:2],
                            op0=OP.subtract, op1=OP.mult)
                    warm(0, 9)
                # preload the exp table while the projections run
                nc.scalar.activation(out=sb_actw[:, 0:1], in_=sb_eps8,
                                     func=AF.Exp)
                warm(0, 8)

            # ---- phase 2: P and V'^T projections -------------------------
            p_t = persist.tile([128, _CCH, _NQ], fp8, tag="P")
            vt_t = persist.tile([128, 32, _C], fp8, tag="VT")

            with (
                tc.tile_pool(name="kq_psum", bufs=2, space="PSUM") as kq_psum,
                tc.tile_pool(name="vt_psum", bufs=2, space="PSUM") as vt_psum,
            ):
                # P[o]: queries only; copies alternate DVE/ACT
                for o in range(_CCH):
                    osl = slice(o * 128, (o + 1) * 128)
                    for jg in range(_NQ // 1024):
                        ps = kq_psum.tile([128, 2, 512], f32, tag="kq")
                        for jj in range(2):
                            j0 = jg * 1024 + jj * 512
                            for p in range(_CCH // 2):
                                nc.tensor.matmul(
                                    ps[:, jj, :],
                                    lhsT=w_tiles["wP"][:, 2 * p:2 * p + 2, osl],
                                    rhs=hn_t[:, 2 * p:2 * p + 2, j0:j0 + 512],
                                    start=(p == 0), stop=(p == _CCH // 2 - 1),
                                    perf_mode=DR)
                        dst = p_t[:, o, jg * 1024:(jg + 1) * 1024]
                        src = ps.rearrange("p a b -> p (a b)")
                        if jg % 2 == 0:
                            nc.vector.tensor_scalar(
                                out=dst, in0=src,
                                scalar1=sb_bPa[:, o:o + 1], scalar2=_SP / _SA,
                                op0=OP.add, op1=OP.mult)
                        else:
                            nc.scalar.activation(
                                out=dst, in_=src, func=AF.Identity,
                                scale=_SP / _SA, bias=sb_bPb[:, o:o + 1])
                # V'^T[j, c]: stationary = hn column slices; two j-tiles per
                # PSUM tile so each evacuation copy moves 1024 columns (the
                # copies, not the matmuls, are the V' throughput limit)
                for jp in range(16):
                    ps2 = vt_psum.tile([128, 2, 512], f32, tag="vt")
                    for jj in range(2):
                        jc = 2 * jp + jj
                        for p in range(_CCH // 2):
                            nc.tensor.matmul(
                                ps2[:, jj, :],
                                lhsT=hn_t[:, 2 * p:2 * p + 2,
                                          jc * 128:(jc + 1) * 128],
                                rhs=w_tiles["wV"][:, 2 * p:2 * p + 2, :],
                                start=(p == 0), stop=(p == _CCH // 2 - 1),
                                perf_mode=DR)
                    dst = vt_t[:, 2 * jp:2 * jp + 2, :].rearrange(
                        "p a b -> p (a b)")
                    src = ps2.rearrange("p a b -> p (a b)")
                    if jp % 2 == 0:
                        nc.scalar.mul(out=dst, in_=src, mul=_SV2 / _SV)
                    else:
                        nc.vector.tensor_scalar_mul(
                            out=dst, in0=src, scalar1=_SV2 / _SV)

            # ---- phase 3: attention + residual, per 512-query block ------
            with (
                tc.tile_pool(name="attw", bufs=1) as attw,
                tc.tile_pool(name="resw", bufs=3) as resw,
                tc.tile_pool(name="s_psum", bufs=2, space="PSUM") as s_psum,
                tc.tile_pool(name="o_psum", bufs=2, space="PSUM") as o_psum,
                tc.tile_pool(name="pd_psum", bufs=2, space="PSUM") as pd_psum,
            ):
                def attn_norm(osb, rbc_sb, cc, isl):
                    """normalize by 1/denom, + (Wp bv + bp) + residual, out.
                    cc 0/2 ride the otherwise-idle Pool engine; cc 1/3 use
                    DVE's fused scalar_tensor_tensor (keeps the last block's
                    tail short -- Pool's queue runs ~10us behind)."""
                    xres = resw.tile([128, 512], f32, tag="xres")
                    nc.sync.dma_start(
                        out=xres, in_=xqf[cc * 128:(cc + 1) * 128, isl])
                    outt = resw.tile([128, 512], f32, tag="outt")
                    if cc % 2 == 0:
                        t1 = resw.tile([128, 512], f32, tag="t1")
                        nc.gpsimd.tensor_tensor(
                            out=t1, in0=osb, in1=rbc_sb, op=OP.mult)
                        o1 = resw.tile([128, 512], f32, tag="o1")
                        nc.gpsimd.tensor_scalar(
                            out=o1, in0=t1, scalar1=sb_bpe[:, cc:cc + 1],
                            scalar2=1.0, op0=OP.add, op1=OP.mult)
                        nc.gpsimd.tensor_tensor(
                            out=outt, in0=o1, in1=xres, op=OP.add)
                        nc.gpsimd.dma_start(
                            out=out_d[cc * 128:(cc + 1) * 128, isl], in_=outt)
                    else:
                        t1 = resw.tile([128, 512], f32, tag="t1")
                        nc.vector.tensor_tensor(
                            out=t1, in0=osb, in1=rbc_sb, op=OP.mult)
                        nc.vector.scalar_tensor_tensor(
                            out=outt, in0=t1, scalar=sb_bpe[:, cc:cc + 1],
                            in1=xres, op0=OP.add, op1=OP.add)
                        nc.sync.dma_start(
                            out=out_d[cc * 128:(cc + 1) * 128, isl], in_=outt)

                for ib in range(_NQ // 512):
                    isl = slice(ib * 512, (ib + 1) * 512)
                    es = attw.tile([128, 32, 512], fp8, tag="ES", bufs=2)
                    l1 = attw.tile([128, 16, 512], bf16, tag="L1")
                    l2 = attw.tile([128, 2, 4, 512], bf16, tag="L2")
                    l3 = attw.tile([128, 2, 2, 512], bf16, tag="L3")
                    dhalf = attw.tile([128, 2, 512], bf16, tag="dh")
                    rbc = pd_psum.tile([128, 512], f32, tag="pd")
                    # scores^T + exp, 2 j-chunks (1024 wide) at a time; the
                    # 1/S_P descale rides the exp's scale immediate.  The
                    # denominator add-tree runs in two halves (contiguous
                    # reads) so only half a tree trails the last exp; the
                    # ones(x16)-matmul accumulates both halves into one PSUM
                    # bank and broadcasts the j-total to all partitions.
                    for jg in range(16):
                        ps = s_psum.tile([128, 2, 512], f32, tag="s")
                        for jj in range(2):
                            jc = jg * 2 + jj
                            for p in range(_CCH // 2):
                                nc.tensor.matmul(
                                    ps[:, jj, :],
                                    lhsT=hn_t[:, 2 * p:2 * p + 2,
                                              jc * 128:(jc + 1) * 128],
                                    rhs=p_t[:, 2 * p:2 * p + 2, isl],
                                    start=(p == 0), stop=(p == _CCH // 2 - 1),
                                    perf_mode=DR)
                        nc.scalar.activation(
                            out=es[:, jg * 2:(jg + 1) * 2, :].rearrange(
                                "p a b -> p (a b)"),
                            in_=ps.rearrange("p a b -> p (a b)"),
                            func=AF.Exp, scale=1.0 / _SP)
                        if jg % 4 == 3:
                            g = jg // 4
                            nc.vector.tensor_tensor(
                                out=l1[:, g * 4:(g + 1) * 4, :],
                                in0=es[:, 8 * g:8 * g + 4, :],
                                in1=es[:, 8 * g + 4:8 * (g + 1), :],
                                op=OP.add)
                        if jg % 8 == 7:
                            hf = jg // 8
                            nc.vector.tensor_tensor(
                                out=l2[:, hf], in0=l1[:, 8 * hf:8 * hf + 4, :],
                                in1=l1[:, 8 * hf + 4:8 * hf + 8, :], op=OP.add)
                            nc.vector.tensor_tensor(
                                out=l3[:, hf], in0=l2[:, hf, 0:2, :],
                                in1=l2[:, hf, 2:4, :], op=OP.add)
                            nc.vector.tensor_tensor(
                                out=dhalf[:, hf, :], in0=l3[:, hf, 0, :],
                                in1=l3[:, hf, 1, :], op=OP.add)
                    # first denominator half: emitted after the S^T loop so
                    # the PE never waits on the DVE tree (dhalf[0] is ready
                    # ~1.5us before the PE drains the S^T matmuls)
                    nc.tensor.matmul(rbc, lhsT=sb_obc16, rhs=dhalf[:, 0, :],
                                     start=True, stop=False)
                    # O^T[c, i] = sum_j V'^T[j,c] expS^T[j,i]: this IS the
                    # (unnormalized) output -- no proj matmul afterwards.
                    # PSUM is evacuated by plain ACT copies so it never waits
                    # on the denominator; Pool/DVE finish the normalize.
                    rbc_sb = attw.tile([128, 512], f32, tag="rbc")
                    for cc in range(_CCH):
                        pso = o_psum.tile([128, 512], f32, tag="o")
                        for jp in range(16):
                            nc.tensor.matmul(
                                pso,
                                lhsT=vt_t[:, 2 * jp:2 * jp + 2,
                                          cc * 128:(cc + 1) * 128],
                                rhs=es[:, 2 * jp:2 * jp + 2, :],
                                start=(jp == 0), stop=(jp == 15),
                                perf_mode=DR)
                        if cc == 0:
                            # second half lands just after AV(cc0) passes it
                            nc.tensor.matmul(
                                rbc, lhsT=sb_obc16, rhs=dhalf[:, 1, :],
                                start=False, stop=True)
                            nc.vector.reciprocal(out=rbc_sb, in_=rbc)
                        osb = resw.tile([128, 512], f32, tag="osb")
                        nc.scalar.copy(out=osb, in_=pso)
                        attn_norm(osb, rbc_sb, cc, isl)

    _legalize_single_wait(nc, mybir)
    return nc


def kernel(**inputs):
    import ml_dtypes
    from concourse.bass_utils import run_bass_kernel_spmd

    global _cached
    if _cached is None:
        _cached = _build_program()
    nc = _cached

    x = np.asarray(inputs["x"], dtype=np.float32)
    gn_w = np.asarray(inputs["gn_w"], dtype=np.float32)
    gn_b = np.asarray(inputs["gn_b"], dtype=np.float32)
    wq = np.asarray(inputs["wq"], dtype=np.float32)
    bq = np.asarray(inputs["bq"], dtype=np.float32)
    wk = np.asarray(inputs["wk"], dtype=np.float32)
    wv = np.asarray(inputs["wv"], dtype=np.float32)
    bv = np.asarray(inputs["bv"], dtype=np.float32)
    wp = np.asarray(inputs["wp"], dtype=np.float32)
    bp = np.asarray(inputs["bp"], dtype=np.float32)
    # bk cancels in the j-softmax (it only adds per-query constants)

    fp8 = ml_dtypes.float8_e4m3
    scale = float(_C) ** -0.5

    def cols(v):  # [512] -> [128, 4] chunk columns
        return np.ascontiguousarray(v.reshape(_CCH, 128).T)

    def wlay(w, s):  # [cout, cin] -> lhsT chunked [128, cch*cout], fp8 x s
        return np.ascontiguousarray(
            w.T.reshape(_CCH, 128, _C).transpose(1, 0, 2).reshape(128, _CCH * _C)
            * s
        ).astype(fp8)

    wP = (wk.T.astype(np.float64) @ wq.astype(np.float64)).astype(np.float32)
    wV = (wp.astype(np.float64) @ wv.astype(np.float64)).astype(np.float32)
    cvec = wk.T @ bq

    consts = np.concatenate([
        cols(cvec * (_SA * scale)),                                 # bPa
        cols(cvec * (_SP * scale)),                                 # bPb
        cols(wp @ bv + bp),                                         # bpe2
        cols(gn_w),                                                 # gnw2
        cols(gn_b),                                                 # gnb2
        np.repeat(np.eye(8, dtype=np.float32), 16, axis=0) / 16.0,  # gmat
    ], axis=1)
    shared = {
        "wPT": wlay(wP, _SA * scale),
        "wVT": wlay(wV, _SV),
        "consts": consts,
        "obc16": np.full((128, 128), _SV2, ml_dtypes.bfloat16),
        "gexp": np.repeat(np.eye(8, dtype=np.float32), 16, axis=1),
    }

    xf = x.reshape(_B, _C, _N)
    in_maps = []
    for core in range(_NCORES):
        bi, qh = core // 2, core % 2
        xbc = xf[bi]
        if qh == 1:  # rotate so this core's queries are columns 0..NQ-1
            xbc = np.concatenate([xbc[:, _NQ:], xbc[:, :_NQ]], axis=1)
        in_maps.append({
            "xb8": np.ascontiguousarray(xbc).astype(fp8),
            "xqf": np.ascontiguousarray(xbc[:, :_NQ], dtype=np.float32),
            **shared,
        })

    res = run_bass_kernel_spmd(nc, in_maps, core_ids=list(range(_NCORES)))

    out = np.empty((_B, _C, _N), np.float32)
    for core in range(_NCORES):
        bi, qh = core // 2, core % 2
        out[bi][:, qh * _NQ:(qh + 1) * _NQ] = res.results[core]["out"]
    return out.reshape(_B, _C, 64, 64)


# revision 21
# speedup vs baseline: 1.2841x; 1.2841x over previous
"""AttnBlock (GroupNorm -> qkv 1x1 -> NxN spatial attention -> proj -> residual)
for Trainium2, SPMD over 8 NeuronCores.

Sharding: core = (batch b in 0..3, query-half qh in 0..1): the host rotates
the spatial columns so each core's 2048 queries are columns 0..2047 of its
input; keys are the full 4096 columns (key order is irrelevant to softmax
attention, and only query columns are written back).

Algebraic restructure vs the straightforward q/k/v/proj pipeline (exact, no
approximation):
  * scores_ij = (Wq hn_i + bq).(Wk hn_j + bk); per-query terms cancel in the
    j-softmax, so scores ~ P_i . hn_j with P = scale*((Wk^T Wq) hn + Wk^T bq).
    The K projection disappears: keys are raw hn, queries get one projection
    by the host-precomputed 512x512 matrix Wk^T Wq.
  * attention rows sum to 1, so the output projection commutes through the
    value sum: out_i = sum_j a_ij (Wp Wv hn)_j + (Wp bv + bp).  V is projected
    once by the host-precomputed Wp Wv; the separate proj matmul disappears.
Both precomputed matrices are weight-only (input-independent), like the bias
folds.  This removes ~25% of the tensor-engine work.

Engine budget per 512-query attention block (PE is the roofline at ~27.5us):
PE S^T 13.7 + AV 13.7; ACT exp 17.8 + psum evacuation 2.7; DVE denominator
tree ~8 + reciprocal 3.4; Pool normalize/bias/residual chain ~16 + out DMA.
The AV psum is evacuated by plain ACT copies so no PSUM buffer ever waits on
the softmax denominator (which needs the full tree + reciprocal); the
denominator is reduced in two halves with the ones-matmul accumulating both
into one PSUM bank, so only half a tree remains after the last exp.

GroupNorm: x is loaded once as fp8 (stats and normalize both read it; the
fp32 residual copy streams in during the attention phase).  Chunk 0 uses
classic sum/sumsq on ACT, chunks 1-3 single-pass BN_STATS on DVE, per-chunk
group chains run interleaved with the later chunks' stats, normalizes split
ACT/DVE.  Engines execute in program order, so ops are emitted in data-arrival
order; the x DMAs are first on both trigger queues.
"""

import numpy as np

_B, _C, _HW = 4, 512, 64 * 64  # batch, channels, spatial N
_N = _HW                       # 4096
_NQ = _N // 2                  # queries per core
_G = 32                        # groupnorm groups
_EPS = 1e-6
_NCORES = 8
_CCH = _C // 128               # 4 channel chunks

# fp8 range scaling (host pre-mult, device descale; all powers of two)
_SA = 4096.0   # on Wk^T Wq entries (~4e-4 std)
_SP = 256.0    # on P in SBUF (~0.01 std)
_SV = 512.0    # on Wp Wv entries (~0.009 std)
_SV2 = 16.0    # on V' in SBUF (~0.2 std); folded back via the ones-matrix

_cached = None  # (nc,) built Bass program, reused across kernel() calls


def _legalize_single_wait(nc, mybir):
    """This container's walrus codegen accepts at most ONE sync-wait per
    instruction. Tile emits N-wait instructions; hoist the extras onto
    injected same-engine NOPs placed immediately before."""
    ctr = 0
    for f in nc.m.functions:
        for bb in f.blocks:
            out = []
            changed = False
            for inst in bb.instructions:
                si = inst.sync_info
                if si is not None and len(si.on_wait) > 1:
                    waits = list(si.on_wait)
                    for w in waits[:-1]:
                        ctr += 1
                        out.append(mybir.InstNoOp(
                            name=f"I-legalize-wait-{ctr}",
                            engine=inst.engine,
                            sync_info=mybir.SyncInfo(on_wait=[w], on_update=[]),
                        ))
                    inst.sync_info = mybir.SyncInfo(
                        on_wait=[waits[-1]], on_update=list(si.on_update))
                    changed = True
                out.append(inst)
            if changed:
                bb.instructions = out


def _build_program():
    import concourse.bass as bass
    import concourse.tile as tile
    import concourse.mybir as mybir

    f32 = mybir.dt.float32
    bf16 = mybir.dt.bfloat16
    fp8 = mybir.dt.float8e4
    DR = mybir.MatmulPerfMode.DoubleRow
    AF = mybir.ActivationFunctionType
    OP = mybir.AluOpType

    nc = bass.Bass(name="attnblock")

    xb8 = nc.declare_dram_parameter("xb8", [_C, _N], fp8, isOutput=False)
    xqf = nc.declare_dram_parameter("xqf", [_C, _NQ], f32, isOutput=False)
    wPT = nc.declare_dram_parameter("wPT", [128, _CCH * _C], fp8, isOutput=False)
    wVT = nc.declare_dram_parameter("wVT", [128, _CCH * _C], fp8, isOutput=False)
    # all small [128, x] f32 constants packed into one tensor:
    # [bPa(4) | bPb(4) | bpe2(4) | gnw2(4) | gnb2(4) | gmat(8)]
    consts = nc.declare_dram_parameter("consts", [128, 28], f32, isOutput=False)
    obc16 = nc.declare_dram_parameter("obc16", [128, 128], bf16, isOutput=False)
    gexp = nc.declare_dram_parameter("gexp", [8, 128], f32, isOutput=False)
    out_d = nc.declare_dram_parameter("out", [_C, _NQ], f32, isOutput=True)

    QW = _N // 4    # 1024: classic-stat quarter
    HW2 = _N // 2

    with tile.TileContext(nc) as tc:
        with (
            tc.tile_pool(name="singles", bufs=1) as singles,
            tc.tile_pool(name="persist", bufs=1) as persist,
        ):
            # ---- input DMAs first on both trigger queues -----------------
            # x (fp8) chunks as SEPARATE half-tiles: DMA-written tiles are
            # waited at tile granularity, so a single whole-chunk tile would
            # stall the first stats op until both halves' queues drained.
            dma_engs = [nc.sync, nc.gpsimd]
            xth = []
            k = 0
            for ci in range(_CCH):
                halves = []
                for h in range(2):
                    xt = persist.tile([128, HW2], fp8, tag=f"xt{ci}_{h}",
                                      name=f"xt{ci}_{h}")
                    eng = dma_engs[k % 2]
                    k += 1
                    sl = slice(h * HW2, (h + 1) * HW2)
                    eng.dma_start(out=xt,
                                  in_=xb8[ci * 128:(ci + 1) * 128, sl])
                    halves.append(xt)
                xth.append(halves)

            # ---- constants / weights -------------------------------------
            sb_consts = singles.tile([128, 28], f32, tag="consts")
            nc.sync.dma_start(out=sb_consts, in_=consts[:, :])
            sb_bPa = sb_consts[:, 0:4]    # S_A*scale*(Wk^T bq)   (DVE copies)
            sb_bPb = sb_consts[:, 4:8]    # S_P*scale*(Wk^T bq)   (ACT copies)
            sb_bpe = sb_consts[:, 8:12]   # Wp bv + bp
            sb_gnw = sb_consts[:, 12:16]
            sb_gnb = sb_consts[:, 16:20]
            sb_gmat = sb_consts[:, 20:28]  # eye-repeat(8)/16
            sb_gexp = singles.tile([8, 128], f32, tag="gexp")
            nc.sync.dma_start(out=sb_gexp, in_=gexp[:, :])
            sb_obc16 = singles.tile([128, 128], bf16, tag="obc16")
            nc.sync.dma_start(out=sb_obc16, in_=obc16[:, :])
            # the two precombined weight matrices (needed ~30us in)
            w_tiles = {}
            for nm, src, eng in (("wP", wPT, nc.sync),
                                 ("wV", wVT, nc.gpsimd)):
                t = singles.tile([128, _CCH, _C], fp8, tag=f"w_{nm}",
                                 name=f"w_{nm}")
                eng.dma_start(
                    out=t, in_=src.rearrange("p (a f) -> p a f", a=_CCH))
                w_tiles[nm] = t

            sb_eps8 = singles.tile([8, 1], f32, tag="eps8")
            nc.vector.memset(sb_eps8, _EPS)
            sb_warm = singles.tile([128, 1], f32, tag="warm1")
            nc.vector.memset(sb_warm, 1.0)
            # ACT table prep: GN needs Sqrt/Square/Identity, all served by
            # the sqrt_and_others table.  Exp is touched once after the last
            # GN ACT op so the exp table load lands during the projections.
            sb_actw = singles.tile([8, 2], f32, tag="actw")
            nc.scalar.activation(out=sb_actw[:, 0:1], in_=sb_eps8, func=AF.Sqrt)
            nc.scalar.activation(out=sb_actw[:, 1:2], in_=sb_eps8,
                                 func=AF.Square)

            # normalize constants per chunk: mu', rstd', -mu'*rstd'
            musig = singles.tile([128, _CCH, 3], f32, tag="musig")

            # hn (normalized x, fp8) packed [c_lo, chunk, N]
            hn_t = persist.tile([128, _CCH, _N], fp8, tag="hn")

            # ---- phase 1: GroupNorm --------------------------------------
            with (
                tc.tile_pool(name="gn_small", bufs=2) as gn_small,
                tc.tile_pool(name="gn_psum", bufs=2, space="PSUM") as gn_psum,
                tc.tile_pool(name="warm_psum", bufs=1, space="PSUM") as warm_psum,
            ):
                # PE warm-up: keep the tensor engine busy through the GN head
                # so the HAM clock is ramped when the projections start.
                warm_ps = warm_psum.tile([128, 512], f32, tag="warm")

                def warm(n_small, n_big):
                    for _ in range(n_small):
                        nc.tensor.matmul(warm_ps[0:1, 0:1], lhsT=sb_warm,
                                         rhs=sb_warm, start=True, stop=True)
                    for _ in range(n_big):
                        nc.tensor.matmul(warm_ps, lhsT=xth[0][0][:, 0:128],
                                         rhs=xth[0][0][:, 0:512],
                                         start=True, stop=True)

                warm(80, 12)

                # per chunk: stats -> group chain -> normalize, interleaved
                # so chunk c's chain runs while chunk c+1's stats stream in.
                # Chunk 0: classic sum/sumsq on ACT (ACT is otherwise idle
                # early); chunks 1-3: single-pass BN_STATS on DVE.
                for ci in range(_CCH):
                    t2 = gn_small.tile([128, 2], f32, tag=f"t2_{ci}")
                    if ci == 0:
                        spart = gn_small.tile([128, 4], f32, tag="spart")
                        qpart = gn_small.tile([128, 4], f32, tag="qpart")
                        for h in range(4):
                            src = xth[ci][h // 2][:, (h % 2) * QW:
                                                  (h % 2 + 1) * QW]
                            qs = slice(h * QW, (h + 1) * QW)
                            nc.scalar.activation(out=hn_t[:, ci, qs],
                                                 in_=src,
                                                 func=AF.Square,
                                                 accum_out=qpart[:, h:h + 1])
                            nc.scalar.activation(out=hn_t[:, ci, qs],
                                                 in_=src,
                                                 func=AF.Identity,
                                                 accum_out=spart[:, h:h + 1])
                        nc.vector.reduce_sum(out=t2[:, 0:1], in_=spart,
                                             axis=mybir.AxisListType.XYZW)
                        nc.vector.reduce_sum(out=t2[:, 1:2], in_=qpart,
                                             axis=mybir.AxisListType.XYZW)
                        nc.vector.tensor_scalar_mul(
                            out=t2[:, 0:1], in0=t2[:, 0:1], scalar1=1.0 / _N)
                        nc.vector.tensor_scalar_mul(
                            out=t2[:, 1:2], in0=t2[:, 1:2], scalar1=1.0 / _N)
                    else:
                        bn8 = gn_small.tile([128, 8, 6], f32, tag=f"bn8_{ci}")
                        for s in range(8):
                            nc.vector.bn_stats(
                                out=bn8[:, s, :],
                                in_=xth[ci][s // 4][:, (s % 4) * 512:
                                                    (s % 4 + 1) * 512])
                        mv = gn_small.tile([128, 2], f32, tag=f"mv_{ci}")
                        nc.vector.bn_aggr(out=mv, in_=bn8)
                        sqm = gn_small.tile([128, 1], f32, tag="sqm")
                        nc.vector.tensor_mul(sqm, mv[:, 0:1], mv[:, 0:1])
                        nc.vector.tensor_copy(out=t2[:, 0:1], in_=mv[:, 0:1])
                        nc.vector.tensor_tensor(
                            out=t2[:, 1:2], in0=mv[:, 1:2], in1=sqm,
                            op=OP.add)
                    # group chain (PSUM copies on DVE so the ACT queue stays
                    # free for the normalizes)
                    pg = gn_psum.tile([8, 2], f32, tag="pg")
                    nc.tensor.matmul(pg, lhsT=sb_gmat, rhs=t2,
                                     start=True, stop=True)
                    gs = gn_small.tile([8, 2], f32, tag="gs")
                    nc.vector.tensor_copy(out=gs, in_=pg)
                    # var_g = m2 - mu^2 ; rstd_g = 1/sqrt(var+eps)
                    musq = gn_small.tile([8, 1], f32, tag="musq")
                    nc.vector.tensor_mul(musq, gs[:, 0:1], gs[:, 0:1])
                    nc.vector.tensor_tensor(
                        out=gs[:, 1:2], in0=gs[:, 1:2], in1=musq,
                        op=OP.subtract)
                    sq8 = gn_small.tile([8, 1], f32, tag="sq8")
                    nc.scalar.activation(
                        out=sq8, in_=gs[:, 1:2], func=AF.Sqrt, bias=sb_eps8)
                    nc.vector.reciprocal(out=gs[:, 1:2], in_=sq8)
                    # broadcast to channels: [128, 2] = gexp.T @ [mu_g, rstd_g]
                    pc = gn_psum.tile([128, 2], f32, tag="pc")
                    nc.tensor.matmul(pc, lhsT=sb_gexp, rhs=gs, start=True,
                                     stop=True)
                    pcs = gn_small.tile([128, 2], f32, tag="pcs")
                    nc.vector.tensor_copy(out=pcs, in_=pc)
                    # fold gamma/beta: rstd' = rstd*gamma; mu' = mu - beta/rstd'
                    nc.vector.tensor_mul(
                        musig[:, ci, 1:2], pcs[:, 1:2], sb_gnw[:, ci:ci + 1])
                    rec = gn_small.tile([128, 1], f32, tag="rec")
                    nc.vector.reciprocal(out=rec, in_=musig[:, ci, 1:2])
                    bs = gn_small.tile([128, 1], f32, tag="bs")
                    nc.vector.tensor_mul(bs, sb_gnb[:, ci:ci + 1], rec)
                    nc.vector.tensor_tensor(
                        out=musig[:, ci, 0:1], in0=pcs[:, 0:1], in1=bs,
                        op=OP.subtract)
                    nc.vector.scalar_tensor_tensor(
                        out=musig[:, ci, 2:3], in0=musig[:, ci, 0:1],
                        scalar=-1.0, in1=musig[:, ci, 1:2],
                        op0=OP.mult, op1=OP.mult)
                    # hn = (x - mu') * rstd' (fp8): chunks 0-2 on ACT (DVE is
                    # streaming later chunks' stats), chunk 3 on DVE (free
                    # right after its chain; DVE fp8 normalize is ~2x ACT's)
                    for h in range(2):
                        hsl = slice(h * HW2, (h + 1) * HW2)
                        if ci < 3:
                            nc.scalar.activation(
                                out=hn_t[:, ci, hsl], in_=xth[ci][h],
                                func=AF.Identity, scale=musig[:, ci, 1:2],
                                bias=musig[:, ci, 2:3])
                        else:
                            nc.vector.tensor_scalar(
                                out=hn_t[:, ci, hsl], in0=xth[ci][h],
                                scalar1=musig[:, ci, 0:1],
                                scalar2=musig[:, ci, 1:2],
                                op0=OP.subtract, op1=OP.mult)
                    warm(0, 9)
                # preload the exp table while the projections run
                nc.scalar.activation(out=sb_actw[:, 0:1], in_=sb_eps8,
                                     func=AF.Exp)
                warm(0, 8)

            # ---- phase 2: P and V'^T projections -------------------------
            p_t = persist.tile([128, _CCH, _NQ], fp8, tag="P")
            vt_t = persist.tile([128, 32, _C], fp8, tag="VT")

            with (
                tc.tile_pool(name="kq_psum", bufs=2, space="PSUM") as kq_psum,
                tc.tile_pool(name="vt_psum", bufs=2, space="PSUM") as vt_psum,
            ):
                # P[o]: queries only; copies alternate DVE/ACT
                for o in range(_CCH):
                    osl = slice(o * 128, (o + 1) * 128)
                    for jg in range(_NQ // 1024):
                        ps = kq_psum.tile([128, 2, 512], f32, tag="kq")
                        for jj in range(2):
                            j0 = jg * 1024 + jj * 512
                            for p in range(_CCH // 2):
                                nc.tensor.matmul(
                                    ps[:, jj, :],
                                    lhsT=w_tiles["wP"][:, 2 * p:2 * p + 2, osl],
                                    rhs=hn_t[:, 2 * p:2 * p + 2, j0:j0 + 512],
                                    start=(p == 0), stop=(p == _CCH // 2 - 1),
                                    perf_mode=DR)
                        dst = p_t[:, o, jg * 1024:(jg + 1) * 1024]
                        src = ps.rearrange("p a b -> p (a b)")
                        if jg % 2 == 0:
                            nc.vector.tensor_scalar(
                                out=dst, in0=src,
                                scalar1=sb_bPa[:, o:o + 1], scalar2=_SP / _SA,
                                op0=OP.add, op1=OP.mult)
                        else:
                            nc.scalar.activation(
                                out=dst, in_=src, func=AF.Identity,
                                scale=_SP / _SA, bias=sb_bPb[:, o:o + 1])
                # V'^T[j, c]: stationary = hn column slices; two j-tiles per
                # PSUM tile so each evacuation copy moves 1024 columns (the
                # copies, not the matmuls, are the V' throughput limit)
                for jp in range(16):
                    ps2 = vt_psum.tile([128, 2, 512], f32, tag="vt")
                    for jj in range(2):
                        jc = 2 * jp + jj
                        for p in range(_CCH // 2):
                            nc.tensor.matmul(
                                ps2[:, jj, :],
                                lhsT=hn_t[:, 2 * p:2 * p + 2,
                                          jc * 128:(jc + 1) * 128],
                                rhs=w_tiles["wV"][:, 2 * p:2 * p + 2, :],
                                start=(p == 0), stop=(p == _CCH // 2 - 1),
                                perf_mode=DR)
                    dst = vt_t[:, 2 * jp:2 * jp + 2, :].rearrange(
                        "p a b -> p (a b)")
                    src = ps2.rearrange("p a b -> p (a b)")
                    if jp % 2 == 0:
                        nc.scalar.mul(out=dst, in_=src, mul=_SV2 / _SV)
                    else:
                        nc.vector.tensor_scalar_mul(
                            out=dst, in0=src, scalar1=_SV2 / _SV)

            # ---- phase 3: attention + residual, per 512-query block ------
            with (
                tc.tile_pool(name="attw", bufs=1) as attw,
                tc.tile_pool(name="resw", bufs=4) as resw,
                tc.tile_pool(name="s_psum", bufs=2, space="PSUM") as s_psum,
                tc.tile_pool(name="o_psum", bufs=2, space="PSUM") as o_psum,
                tc.tile_pool(name="pd_psum", bufs=2, space="PSUM") as pd_psum,
            ):
                def attn_norm(osb, rbc_sb, cc, isl):
                    """normalize by 1/denom, + (Wp bv + bp) + residual, out.
                    cc 0/2 ride the otherwise-idle Pool engine; cc 1/3 use
                    DVE's fused scalar_tensor_tensor (keeps the last block's
                    tail short -- Pool's queue runs ~10us behind)."""
                    xres = resw.tile([128, 512], f32, tag="xres")
                    nc.sync.dma_start(
                        out=xres, in_=xqf[cc * 128:(cc + 1) * 128, isl])
                    outt = resw.tile([128, 512], f32, tag="outt")
                    if cc % 2 == 0:
                        t1 = resw.tile([128, 512], f32, tag="t1")
                        nc.gpsimd.tensor_tensor(
                            out=t1, in0=osb, in1=rbc_sb, op=OP.mult)
                        o1 = resw.tile([128, 512], f32, tag="o1")
                        nc.gpsimd.tensor_scalar(
                            out=o1, in0=t1, scalar1=sb_bpe[:, cc:cc + 1],
                            scalar2=1.0, op0=OP.add, op1=OP.mult)
                        nc.gpsimd.tensor_tensor(
                            out=outt, in0=o1, in1=xres, op=OP.add)
                        nc.gpsimd.dma_start(
                            out=out_d[cc * 128:(cc + 1) * 128, isl], in_=outt)
                    else:
                        t1 = resw.tile([128, 512], f32, tag="t1")
                        nc.vector.tensor_tensor(
                            out=t1, in0=osb, in1=rbc_sb, op=OP.mult)
                        nc.vector.scalar_tensor_tensor(
                            out=outt, in0=t1, scalar=sb_bpe[:, cc:cc + 1],
                            in1=xres, op0=OP.add, op1=OP.add)
                        nc.sync.dma_start(
                            out=out_d[cc * 128:(cc + 1) * 128, isl], in_=outt)

                for ib in range(_NQ // 512):
                    isl = slice(ib * 512, (ib + 1) * 512)
                    es = attw.tile([128, 32, 512], fp8, tag="ES", bufs=2)
                    l1 = attw.tile([128, 16, 512], bf16, tag="L1")
                    l2 = attw.tile([128, 2, 4, 512], bf16, tag="L2")
                    l3 = attw.tile([128, 2, 2, 512], bf16, tag="L3")
                    dhalf = attw.tile([128, 2, 512], bf16, tag="dh")
                    rbc = pd_psum.tile([128, 512], f32, tag="pd")
                    # The S^T matmuls alone outrun the exps (PSUM allows only
                    # 2 tiles in flight, so the PE would stall at the exps'
                    # pace); interleaving the AV accumulation for channel
                    # chunks 0/1 (whose es inputs are 2 groups behind) keeps
                    # the PE fed at exactly the rate ACT can sustain.
                    # Denominator add-tree in two halves (contiguous reads);
                    # the ones(x16)-matmul accumulates both halves into one
                    # PSUM bank and broadcasts the j-total to all partitions.
                    av01 = [o_psum.tile([128, 512], f32, tag="o",
                                        name=f"av{ib}_{c}")
                            for c in range(2)]

                    def av_mm(cc, jp, pso):
                        nc.tensor.matmul(
                            pso,
                            lhsT=vt_t[:, 2 * jp:2 * jp + 2,
                                      cc * 128:(cc + 1) * 128],
                            rhs=es[:, 2 * jp:2 * jp + 2, :],
                            start=(jp == 0), stop=(jp == 15),
                            perf_mode=DR)

                    for jg in range(16):
                        ps = s_psum.tile([128, 2, 512], f32, tag="s")
                        for jj in range(2):
                            jc = jg * 2 + jj
                            for p in range(_CCH // 2):
                                nc.tensor.matmul(
                                    ps[:, jj, :],
                                    lhsT=hn_t[:, 2 * p:2 * p + 2,
                                              jc * 128:(jc + 1) * 128],
                                    rhs=p_t[:, 2 * p:2 * p + 2, isl],
                                    start=(p == 0), stop=(p == _CCH // 2 - 1),
                                    perf_mode=DR)
                        if jg >= 2:
                            av_mm(0, jg - 2, av01[0])
                            av_mm(1, jg - 2, av01[1])
                        nc.scalar.activation(
                            out=es[:, jg * 2:(jg + 1) * 2, :].rearrange(
                                "p a b -> p (a b)"),
                            in_=ps.rearrange("p a b -> p (a b)"),
                            func=AF.Exp, scale=1.0 / _SP)
                        if jg % 4 == 3:
                            g = jg // 4
                            nc.vector.tensor_tensor(
                                out=l1[:, g * 4:(g + 1) * 4, :],
                                in0=es[:, 8 * g:8 * g + 4, :],
                                in1=es[:, 8 * g + 4:8 * (g + 1), :],
                                op=OP.add)
                        if jg % 8 == 7:
                            hf = jg // 8
                            nc.vector.tensor_tensor(
                                out=l2[:, hf], in0=l1[:, 8 * hf:8 * hf + 4, :],
                                in1=l1[:, 8 * hf + 4:8 * hf + 8, :], op=OP.add)
                            nc.vector.tensor_tensor(
                                out=l3[:, hf], in0=l2[:, hf, 0:2, :],
                                in1=l2[:, hf, 2:4, :], op=OP.add)
                            nc.vector.tensor_tensor(
                                out=dhalf[:, hf, :], in0=l3[:, hf, 0, :],
                                in1=l3[:, hf, 1, :], op=OP.add)
                    for jp in (14, 15):
                        av_mm(0, jp, av01[0])
                        av_mm(1, jp, av01[1])
                    # denominator halves: dhalf[0] has been ready since
                    # mid-loop; dhalf[1] lands ~2us after the last exp, just
                    # as the PE drains the trailing AV matmuls
                    nc.tensor.matmul(rbc, lhsT=sb_obc16, rhs=dhalf[:, 0, :],
                                     start=True, stop=False)
                    nc.tensor.matmul(rbc, lhsT=sb_obc16, rhs=dhalf[:, 1, :],
                                     start=False, stop=True)
                    rbc_sb = attw.tile([128, 512], f32, tag="rbc")
                    nc.vector.reciprocal(out=rbc_sb, in_=rbc)
                    # O^T[c, i] = sum_j V'^T[j,c] expS^T[j,i]: this IS the
                    # (unnormalized) output -- no proj matmul afterwards.
                    # PSUM is evacuated by plain ACT copies so it never waits
                    # on the denominator; Pool/DVE finish the normalize.
                    for cc in (0, 1):
                        osb = resw.tile([128, 512], f32, tag="osb")
                        nc.scalar.copy(out=osb, in_=av01[cc])
                        attn_norm(osb, rbc_sb, cc, isl)
                    for cc in (2, 3):
                        pso = o_psum.tile([128, 512], f32, tag="o")
                        for jp in range(16):
                            av_mm(cc, jp, pso)
                        osb = resw.tile([128, 512], f32, tag="osb")
                        nc.scalar.copy(out=osb, in_=pso)
                        attn_norm(osb, rbc_sb, cc, isl)

    _legalize_single_wait(nc, mybir)
    return nc


def kernel(**inputs):
    import ml_dtypes
    from concourse.bass_utils import run_bass_kernel_spmd

    global _cached
    if _cached is None:
        _cached = _build_program()
    nc = _cached

    x = np.asarray(inputs["x"], dtype=np.float32)
    gn_w = np.asarray(inputs["gn_w"], dtype=np.float32)
    gn_b = np.asarray(inputs["gn_b"], dtype=np.float32)
    wq = np.asarray(inputs["wq"], dtype=np.float32)
    bq = np.asarray(inputs["bq"], dtype=np.float32)
    wk = np.asarray(inputs["wk"], dtype=np.float32)
    wv = np.asarray(inputs["wv"], dtype=np.float32)
    bv = np.asarray(inputs["bv"], dtype=np.float32)
    wp = np.asarray(inputs["wp"], dtype=np.float32)
    bp = np.asarray(inputs["bp"], dtype=np.float32)
    # bk cancels in the j-softmax (it only adds per-query constants)

    fp8 = ml_dtypes.float8_e4m3
    scale = float(_C) ** -0.5

    def cols(v):  # [512] -> [128, 4] chunk columns
        return np.ascontiguousarray(v.reshape(_CCH, 128).T)

    def wlay(w, s):  # [cout, cin] -> lhsT chunked [128, cch*cout], fp8 x s
        return np.ascontiguousarray(
            w.T.reshape(_CCH, 128, _C).transpose(1, 0, 2).reshape(128, _CCH * _C)
            * s
        ).astype(fp8)

    wP = (wk.T.astype(np.float64) @ wq.astype(np.float64)).astype(np.float32)
    wV = (wp.astype(np.float64) @ wv.astype(np.float64)).astype(np.float32)
    cvec = wk.T @ bq

    consts = np.concatenate([
        cols(cvec * (_SA * scale)),                                 # bPa
        cols(cvec * (_SP * scale)),                                 # bPb
        cols(wp @ bv + bp),                                         # bpe2
        cols(gn_w),                                                 # gnw2
        cols(gn_b),                                                 # gnb2
        np.repeat(np.eye(8, dtype=np.float32), 16, axis=0) / 16.0,  # gmat
    ], axis=1)
    shared = {
        "wPT": wlay(wP, _SA * scale),
        "wVT": wlay(wV, _SV),
        "consts": consts,
        "obc16": np.full((128, 128), _SV2, ml_dtypes.bfloat16),
        "gexp": np.repeat(np.eye(8, dtype=np.float32), 16, axis=1),
    }

    xf = x.reshape(_B, _C, _N)
    in_maps = []
    for core in range(_NCORES):
        bi, qh = core // 2, core % 2
        xbc = xf[bi]
        if qh == 1:  # rotate so this core's queries are columns 0..NQ-1
            xbc = np.concatenate([xbc[:, _NQ:], xbc[:, :_NQ]], axis=1)
        in_maps.append({
            "xb8": np.ascontiguousarray(xbc).astype(fp8),
            "xqf": np.ascontiguousarray(xbc[:, :_NQ], dtype=np.float32),
            **shared,
        })

    res = run_bass_kernel_spmd(nc, in_maps, core_ids=list(range(_NCORES)))

    out = np.empty((_B, _C, _N), np.float32)
    for core in range(_NCORES):
        bi, qh = core // 2, core % 2
        out[bi][:, qh * _NQ:(qh + 1) * _NQ] = res.results[core]["out"]
    return out.reshape(_B, _C, 64, 64)


# revision 30
# speedup vs baseline: 1.3536x; 1.0541x over previous
"""AttnBlock (GroupNorm -> qkv 1x1 -> NxN spatial attention -> proj -> residual)
for Trainium2, SPMD over 8 NeuronCores.

Sharding: core = (batch b in 0..3, query-half qh in 0..1): the host rotates
the spatial columns so each core's 2048 queries are columns 0..2047 of its
input; keys are the full 4096 columns (key order is irrelevant to softmax
attention, and only query columns are written back).

Algebraic restructure vs the straightforward q/k/v/proj pipeline (exact, no
approximation):
  * scores_ij = (Wq hn_i + bq).(Wk hn_j + bk); per-query terms cancel in the
    j-softmax, so scores ~ P_i . hn_j with P = scale*((Wk^T Wq) hn + Wk^T bq).
    The K projection disappears: keys are raw hn, queries get one projection
    by the host-precomputed 512x512 matrix Wk^T Wq.
  * attention rows sum to 1, so the output projection commutes through the
    value sum: out_i = sum_j a_ij (Wp Wv hn)_j + (Wp bv + bp).  V is projected
    once by the host-precomputed Wp Wv; the separate proj matmul disappears.
Both precomputed matrices are weight-only (input-independent), like the bias
folds.  This removes ~25% of the tensor-engine work.

Engine budget per 512-query attention block (PE is the roofline at ~27.5us):
PE S^T 13.7 + AV 13.7; ACT exp 17.8 + psum evacuation 2.7; DVE denominator
tree ~8 + reciprocal 3.4; Pool normalize/bias/residual chain ~16 + out DMA.
The AV psum is evacuated by plain ACT copies so no PSUM buffer ever waits on
the softmax denominator (which needs the full tree + reciprocal); the
denominator is reduced in two halves with the ones-matmul accumulating both
into one PSUM bank, so only half a tree remains after the last exp.

GroupNorm: x is loaded once as fp8 (stats and normalize both read it; the
fp32 residual copy streams in during the attention phase).  Chunk 0 uses
classic sum/sumsq on ACT, chunks 1-3 single-pass BN_STATS on DVE, per-chunk
group chains run interleaved with the later chunks' stats, normalizes split
ACT/DVE.  Engines execute in program order, so ops are emitted in data-arrival
order; the x DMAs are first on both trigger queues.
"""

import numpy as np

_B, _C, _HW = 4, 512, 64 * 64  # batch, channels, spatial N
_N = _HW                       # 4096
_NQ = _N // 2                  # queries per core
_G = 32                        # groupnorm groups
_EPS = 1e-6
_NCORES = 8
_CCH = _C // 128               # 4 channel chunks

# fp8 range scaling (host pre-mult, device descale; all powers of two)
_SA = 4096.0   # on Wk^T Wq entries (~4e-4 std)
_SP = 256.0    # on P in SBUF (~0.01 std)
_SV = 512.0    # on Wp Wv entries (~0.009 std)
_SV2 = 16.0    # on V' in SBUF (~0.2 std); folded back via the ones-matrix

_cached = None  # (nc,) built Bass program, reused across kernel() calls


def _legalize_single_wait(nc, mybir):
    """This container's walrus codegen accepts at most ONE sync-wait per
    instruction. Tile emits N-wait instructions; hoist the extras onto
    injected same-engine NOPs placed immediately before."""
    ctr = 0
    for f in nc.m.functions:
        for bb in f.blocks:
            out = []
            changed = False
            for inst in bb.instructions:
                si = inst.sync_info
                if si is not None and len(si.on_wait) > 1:
                    waits = list(si.on_wait)
                    for w in waits[:-1]:
                        ctr += 1
                        out.append(mybir.InstNoOp(
                            name=f"I-legalize-wait-{ctr}",
                            engine=inst.engine,
                            sync_info=mybir.SyncInfo(on_wait=[w], on_update=[]),
                        ))
                    inst.sync_info = mybir.SyncInfo(
                        on_wait=[waits[-1]], on_update=list(si.on_update))
                    changed = True
                out.append(inst)
            if changed:
                bb.instructions = out


def _build_program():
    import concourse.bass as bass
    import concourse.tile as tile
    import concourse.mybir as mybir

    f32 = mybir.dt.float32
    bf16 = mybir.dt.bfloat16
    fp8 = mybir.dt.float8e4
    DR = mybir.MatmulPerfMode.DoubleRow
    AF = mybir.ActivationFunctionType
    OP = mybir.AluOpType

    nc = bass.Bass(name="attnblock")

    xb8 = nc.declare_dram_parameter("xb8", [_C, _N], fp8, isOutput=False)
    xqf = nc.declare_dram_parameter("xqf", [_C, _NQ], f32, isOutput=False)
    wPT = nc.declare_dram_parameter("wPT", [128, _CCH * _C], fp8, isOutput=False)
    wVT = nc.declare_dram_parameter("wVT", [128, _CCH * _C], fp8, isOutput=False)
    # all small [128, x] f32 constants packed into one tensor:
    # [bPa(4) | bPb(4) | bpe2(4) | gnw2(4) | gnb2(4) | gmat(8)]
    consts = nc.declare_dram_parameter("consts", [128, 28], f32, isOutput=False)
    obc16 = nc.declare_dram_parameter("obc16", [128, 128], bf16, isOutput=False)
    obc8 = nc.declare_dram_parameter("obc8", [128, 256], fp8, isOutput=False)
    gexp = nc.declare_dram_parameter("gexp", [8, 128], f32, isOutput=False)
    out_d = nc.declare_dram_parameter("out", [_C, _NQ], f32, isOutput=True)

    QW = _N // 4    # 1024: classic-stat quarter
    HW2 = _N // 2

    with tile.TileContext(nc) as tc:
        with (
            tc.tile_pool(name="singles", bufs=1) as singles,
            tc.tile_pool(name="persist", bufs=1) as persist,
        ):
            # ---- input DMAs first on both trigger queues -----------------
            # x (fp8) chunks as SEPARATE half-tiles: DMA-written tiles are
            # waited at tile granularity, so a single whole-chunk tile would
            # stall the first stats op until both halves' queues drained.
            dma_engs = [nc.sync, nc.gpsimd]
            xth = []
            k = 0
            for ci in range(_CCH):
                halves = []
                for h in range(2):
                    xt = persist.tile([128, HW2], fp8, tag=f"xt{ci}_{h}",
                                      name=f"xt{ci}_{h}")
                    eng = dma_engs[k % 2]
                    k += 1
                    sl = slice(h * HW2, (h + 1) * HW2)
                    eng.dma_start(out=xt,
                                  in_=xb8[ci * 128:(ci + 1) * 128, sl])
                    halves.append(xt)
                xth.append(halves)

            # ---- constants / weights -------------------------------------
            sb_consts = singles.tile([128, 28], f32, tag="consts")
            nc.sync.dma_start(out=sb_consts, in_=consts[:, :])
            sb_bPa = sb_consts[:, 0:4]    # S_A*scale*(Wk^T bq)   (DVE copies)
            sb_bPb = sb_consts[:, 4:8]    # S_P*scale*(Wk^T bq)   (ACT copies)
            sb_bpe = sb_consts[:, 8:12]   # Wp bv + bp
            sb_gnw = sb_consts[:, 12:16]
            sb_gnb = sb_consts[:, 16:20]
            sb_gmat = sb_consts[:, 20:28]  # eye-repeat(8)/16
            sb_gexp = singles.tile([8, 128], f32, tag="gexp")
            nc.sync.dma_start(out=sb_gexp, in_=gexp[:, :])
            sb_obc16 = singles.tile([128, 128], bf16, tag="obc16")
            nc.sync.dma_start(out=sb_obc16, in_=obc16[:, :])
            sb_ones8 = singles.tile([128, 2, 128], fp8, tag="ones8")
            nc.sync.dma_start(
                out=sb_ones8, in_=obc8.rearrange("p (a f) -> p a f", a=2))
            # the two precombined weight matrices (needed ~30us in)
            w_tiles = {}
            for nm, src, eng in (("wP", wPT, nc.sync),
                                 ("wV", wVT, nc.gpsimd)):
                t = singles.tile([128, _CCH, _C], fp8, tag=f"w_{nm}",
                                 name=f"w_{nm}")
                eng.dma_start(
                    out=t, in_=src.rearrange("p (a f) -> p a f", a=_CCH))
                w_tiles[nm] = t

            sb_eps8 = singles.tile([8, 1], f32, tag="eps8")
            nc.vector.memset(sb_eps8, _EPS)
            sb_warm = singles.tile([128, 1], f32, tag="warm1")
            nc.vector.memset(sb_warm, 1.0)
            # ACT table prep: GN needs Sqrt/Square/Identity, all served by
            # the sqrt_and_others table.  Exp is touched once after the last
            # GN ACT op so the exp table load lands during the projections.
            sb_actw = singles.tile([8, 2], f32, tag="actw")
            nc.scalar.activation(out=sb_actw[:, 0:1], in_=sb_eps8, func=AF.Sqrt)
            nc.scalar.activation(out=sb_actw[:, 1:2], in_=sb_eps8,
                                 func=AF.Square)

            # normalize constants per chunk: mu', rstd', -mu'*rstd'
            musig = singles.tile([128, _CCH, 3], f32, tag="musig")

            # hn (normalized x, fp8) packed [c_lo, chunk, N]
            hn_t = persist.tile([128, _CCH, _N], fp8, tag="hn")

            # ---- phase 1: GroupNorm --------------------------------------
            with (
                tc.tile_pool(name="gn_small", bufs=2) as gn_small,
                tc.tile_pool(name="gn_psum", bufs=2, space="PSUM") as gn_psum,
                tc.tile_pool(name="warm_psum", bufs=1, space="PSUM") as warm_psum,
            ):
                # PE warm-up: keep the tensor engine busy through the GN head
                # so the HAM clock is ramped when the projections start.
                warm_ps = warm_psum.tile([128, 512], f32, tag="warm")

                def warm(n_small, n_big):
                    for _ in range(n_small):
                        nc.tensor.matmul(warm_ps[0:1, 0:1], lhsT=sb_warm,
                                         rhs=sb_warm, start=True, stop=True)
                    for _ in range(n_big):
                        nc.tensor.matmul(warm_ps, lhsT=xth[0][0][:, 0:128],
                                         rhs=xth[0][0][:, 0:512],
                                         start=True, stop=True)

                warm(80, 12)

                # ALL per-chunk stats first, then all chains: the chains hop
                # engines (PE/ACT round trips), and anything emitted between
                # two chunks' stats stalls the DVE queue for multiple us.
                # Chunk 0: classic sum/sumsq on ACT (ACT is otherwise idle
                # early); chunks 1-3: single-pass BN_STATS on DVE.
                t2s = []
                for ci in range(_CCH):
                    t2 = gn_small.tile([128, 2], f32, tag=f"t2_{ci}")
                    t2s.append(t2)
                    if ci == 0:
                        spart = gn_small.tile([128, 4], f32, tag="spart")
                        qpart = gn_small.tile([128, 4], f32, tag="qpart")
                        for h in range(4):
                            src = xth[ci][h // 2][:, (h % 2) * QW:
                                                  (h % 2 + 1) * QW]
                            qs = slice(h * QW, (h + 1) * QW)
                            nc.scalar.activation(out=hn_t[:, ci, qs],
                                                 in_=src,
                                                 func=AF.Square,
                                                 accum_out=qpart[:, h:h + 1])
                            nc.scalar.activation(out=hn_t[:, ci, qs],
                                                 in_=src,
                                                 func=AF.Identity,
                                                 accum_out=spart[:, h:h + 1])
                        nc.vector.reduce_sum(out=t2[:, 0:1], in_=spart,
                                             axis=mybir.AxisListType.XYZW)
                        nc.vector.reduce_sum(out=t2[:, 1:2], in_=qpart,
                                             axis=mybir.AxisListType.XYZW)
                        nc.vector.tensor_scalar_mul(
                            out=t2[:, 0:1], in0=t2[:, 0:1], scalar1=1.0 / _N)
                        nc.vector.tensor_scalar_mul(
                            out=t2[:, 1:2], in0=t2[:, 1:2], scalar1=1.0 / _N)
                    else:
                        bn8 = gn_small.tile([128, 8, 6], f32, tag=f"bn8_{ci}")
                        for s in range(8):
                            nc.vector.bn_stats(
                                out=bn8[:, s, :],
                                in_=xth[ci][s // 4][:, (s % 4) * 512:
                                                    (s % 4 + 1) * 512])
                        mv = gn_small.tile([128, 2], f32, tag=f"mv_{ci}")
                        nc.vector.bn_aggr(out=mv, in_=bn8)
                        sqm = gn_small.tile([128, 1], f32, tag="sqm")
                        nc.vector.tensor_mul(sqm, mv[:, 0:1], mv[:, 0:1])
                        nc.vector.tensor_copy(out=t2[:, 0:1], in_=mv[:, 0:1])
                        nc.vector.tensor_tensor(
                            out=t2[:, 1:2], in0=mv[:, 1:2], in1=sqm,
                            op=OP.add)
                    warm(0, 7)

                for ci in range(_CCH):
                    # group chain (PSUM copies on DVE so the ACT queue stays
                    # free for the normalizes)
                    pg = gn_psum.tile([8, 2], f32, tag="pg")
                    nc.tensor.matmul(pg, lhsT=sb_gmat, rhs=t2s[ci],
                                     start=True, stop=True)
                    gs = gn_small.tile([8, 2], f32, tag="gs")
                    nc.vector.tensor_copy(out=gs, in_=pg)
                    # var_g = m2 - mu^2 ; rstd_g = 1/sqrt(var+eps)
                    musq = gn_small.tile([8, 1], f32, tag="musq")
                    nc.vector.tensor_mul(musq, gs[:, 0:1], gs[:, 0:1])
                    nc.vector.tensor_tensor(
                        out=gs[:, 1:2], in0=gs[:, 1:2], in1=musq,
                        op=OP.subtract)
                    sq8 = gn_small.tile([8, 1], f32, tag="sq8")
                    nc.scalar.activation(
                        out=sq8, in_=gs[:, 1:2], func=AF.Sqrt, bias=sb_eps8)
                    nc.vector.reciprocal(out=gs[:, 1:2], in_=sq8)
                    # broadcast to channels: [128, 2] = gexp.T @ [mu_g, rstd_g]
                    pc = gn_psum.tile([128, 2], f32, tag="pc")
                    nc.tensor.matmul(pc, lhsT=sb_gexp, rhs=gs, start=True,
                                     stop=True)
                    pcs = gn_small.tile([128, 2], f32, tag="pcs")
                    nc.vector.tensor_copy(out=pcs, in_=pc)
                    # fold gamma/beta: rstd' = rstd*gamma; mu' = mu - beta/rstd'
                    nc.vector.tensor_mul(
                        musig[:, ci, 1:2], pcs[:, 1:2], sb_gnw[:, ci:ci + 1])
                    rec = gn_small.tile([128, 1], f32, tag="rec")
                    nc.vector.reciprocal(out=rec, in_=musig[:, ci, 1:2])
                    bs = gn_small.tile([128, 1], f32, tag="bs")
                    nc.vector.tensor_mul(bs, sb_gnb[:, ci:ci + 1], rec)
                    nc.vector.tensor_tensor(
                        out=musig[:, ci, 0:1], in0=pcs[:, 0:1], in1=bs,
                        op=OP.subtract)
                    nc.vector.scalar_tensor_tensor(
                        out=musig[:, ci, 2:3], in0=musig[:, ci, 0:1],
                        scalar=-1.0, in1=musig[:, ci, 1:2],
                        op0=OP.mult, op1=OP.mult)
                    # hn = (x - mu') * rstd' (fp8): chunks 0-2 on ACT (DVE is
                    # streaming later chunks' stats), chunk 3 on DVE (free
                    # right after its chain; DVE fp8 normalize is ~2x ACT's)
                    for h in range(2):
                        hsl = slice(h * HW2, (h + 1) * HW2)
                        if ci < 3:
                            nc.scalar.activation(
                                out=hn_t[:, ci, hsl], in_=xth[ci][h],
                                func=AF.Identity, scale=musig[:, ci, 1:2],
                                bias=musig[:, ci, 2:3])
                        else:
                            nc.vector.tensor_scalar(
                                out=hn_t[:, ci, hsl], in0=xth[ci][h],
                                scalar1=musig[:, ci, 0:1],
                                scalar2=musig[:, ci, 1:2],
                                op0=OP.subtract, op1=OP.mult)
                    warm(0, 4)
                # preload the exp table while the projections run
                nc.scalar.activation(out=sb_actw[:, 0:1], in_=sb_eps8,
                                     func=AF.Exp)
                warm(0, 8)

            # ---- phase 2: P and V'^T projections -------------------------
            p_t = persist.tile([128, _CCH, _NQ], fp8, tag="P")
            vt_t = persist.tile([128, 32, _C], fp8, tag="VT")

            with (
                tc.tile_pool(name="kq_psum", bufs=2, space="PSUM") as kq_psum,
                tc.tile_pool(name="vt_psum", bufs=2, space="PSUM") as vt_psum,
            ):
                # P[o]: queries only; copies alternate DVE/ACT
                for o in range(_CCH):
                    osl = slice(o * 128, (o + 1) * 128)
                    for jg in range(_NQ // 1024):
                        ps = kq_psum.tile([128, 2, 512], f32, tag="kq")
                        for jj in range(2):
                            j0 = jg * 1024 + jj * 512
                            for p in range(_CCH // 2):
                                nc.tensor.matmul(
                                    ps[:, jj, :],
                                    lhsT=w_tiles["wP"][:, 2 * p:2 * p + 2, osl],
                                    rhs=hn_t[:, 2 * p:2 * p + 2, j0:j0 + 512],
                                    start=(p == 0), stop=(p == _CCH // 2 - 1),
                                    perf_mode=DR)
                        dst = p_t[:, o, jg * 1024:(jg + 1) * 1024]
                        src = ps.rearrange("p a b -> p (a b)")
                        if jg % 2 == 0:
                            nc.vector.tensor_scalar(
                                out=dst, in0=src,
                                scalar1=sb_bPa[:, o:o + 1], scalar2=_SP / _SA,
                                op0=OP.add, op1=OP.mult)
                        else:
                            nc.scalar.activation(
                                out=dst, in_=src, func=AF.Identity,
                                scale=_SP / _SA, bias=sb_bPb[:, o:o + 1])
                # V'^T[j, c]: stationary = hn column slices; two j-tiles per
                # PSUM tile so each evacuation copy moves 1024 columns (the
                # copies, not the matmuls, are the V' throughput limit)
                for jp in range(16):
                    ps2 = vt_psum.tile([128, 2, 512], f32, tag="vt")
                    for jj in range(2):
                        jc = 2 * jp + jj
                        for p in range(_CCH // 2):
                            nc.tensor.matmul(
                                ps2[:, jj, :],
                                lhsT=hn_t[:, 2 * p:2 * p + 2,
                                          jc * 128:(jc + 1) * 128],
                                rhs=w_tiles["wV"][:, 2 * p:2 * p + 2, :],
                                start=(p == 0), stop=(p == _CCH // 2 - 1),
                                perf_mode=DR)
                    dst = vt_t[:, 2 * jp:2 * jp + 2, :].rearrange(
                        "p a b -> p (a b)")
                    src = ps2.rearrange("p a b -> p (a b)")
                    if jp % 2 == 0:
                        nc.scalar.mul(out=dst, in_=src, mul=_SV2 / _SV)
                    else:
                        nc.vector.tensor_scalar_mul(
                            out=dst, in0=src, scalar1=_SV2 / _SV)

            # ---- phase 3: attention + residual, per 512-query block ------
            with (
                tc.tile_pool(name="attw", bufs=1) as attw,
                tc.tile_pool(name="resw", bufs=4) as resw,
                tc.tile_pool(name="s_psum", bufs=2, space="PSUM") as s_psum,
                tc.tile_pool(name="o_psum", bufs=2, space="PSUM") as o_psum,
                tc.tile_pool(name="pd_psum", bufs=2, space="PSUM") as pd_psum,
            ):
                def attn_norm(osb, rbc_sb, cc, isl, on_pool):
                    """normalize by 1/denom, + (Wp bv + bp) + residual, out.
                    Half the chains ride the otherwise-idle Pool engine; the
                    rest use DVE's fused scalar_tensor_tensor (also: all of
                    the last block's, since Pool ops run ~1.3us each and
                    would stretch the kernel's tail)."""
                    xres = resw.tile([128, 512], f32, tag="xres")
                    nc.sync.dma_start(
                        out=xres, in_=xqf[cc * 128:(cc + 1) * 128, isl])
                    outt = resw.tile([128, 512], f32, tag="outt")
                    if on_pool:
                        t1 = resw.tile([128, 512], f32, tag="t1")
                        nc.gpsimd.tensor_tensor(
                            out=t1, in0=osb, in1=rbc_sb, op=OP.mult)
                        o1 = resw.tile([128, 512], f32, tag="o1")
                        nc.gpsimd.tensor_scalar(
                            out=o1, in0=t1, scalar1=sb_bpe[:, cc:cc + 1],
                            scalar2=1.0, op0=OP.add, op1=OP.mult)
                        nc.gpsimd.tensor_tensor(
                            out=outt, in0=o1, in1=xres, op=OP.add)
                        nc.gpsimd.dma_start(
                            out=out_d[cc * 128:(cc + 1) * 128, isl], in_=outt)
                    else:
                        t1 = resw.tile([128, 512], f32, tag="t1")
                        nc.vector.tensor_tensor(
                            out=t1, in0=osb, in1=rbc_sb, op=OP.mult)
                        nc.vector.scalar_tensor_tensor(
                            out=outt, in0=t1, scalar=sb_bpe[:, cc:cc + 1],
                            in1=xres, op0=OP.add, op1=OP.add)
                        nc.sync.dma_start(
                            out=out_d[cc * 128:(cc + 1) * 128, isl], in_=outt)

                for ib in range(_NQ // 512):
                    last = ib == (_NQ // 512) - 1
                    isl = slice(ib * 512, (ib + 1) * 512)
                    es = attw.tile([128, 32, 512], fp8, tag="ES", bufs=2)
                    l1 = attw.tile([128, 16, 512], bf16, tag="L1")
                    l2 = attw.tile([128, 2, 4, 512], bf16, tag="L2")
                    l3 = attw.tile([128, 2, 2, 512], bf16, tag="L3")
                    dhalf = attw.tile([128, 2, 512], bf16, tag="dh")
                    rbc = pd_psum.tile([128, 512], f32, tag="pd")
                    # The S^T matmuls alone outrun the exps (PSUM allows only
                    # 2 tiles in flight, so the PE would stall at the exps'
                    # pace); interleaving the AV accumulation for channel
                    # chunks 0/1 (whose es inputs are 2 groups behind) keeps
                    # the PE fed at exactly the rate ACT can sustain.
                    # Denominator add-tree in two halves (contiguous reads);
                    # the ones(x16)-matmul accumulates both halves into one
                    # PSUM bank and broadcasts the j-total to all partitions.
                    av01 = [o_psum.tile([128, 512], f32, tag="o",
                                        name=f"av{ib}_{c}")
                            for c in range(2)]

                    def av_mm(cc, jp, pso):
                        nc.tensor.matmul(
                            pso,
                            lhsT=vt_t[:, 2 * jp:2 * jp + 2,
                                      cc * 128:(cc + 1) * 128],
                            rhs=es[:, 2 * jp:2 * jp + 2, :],
                            start=(jp == 0), stop=(jp == 15),
                            perf_mode=DR)

                    for jg in range(16):
                        ps = s_psum.tile([128, 2, 512], f32, tag="s")
                        for jj in range(2):
                            jc = jg * 2 + jj
                            for p in range(_CCH // 2):
                                nc.tensor.matmul(
                                    ps[:, jj, :],
                                    lhsT=hn_t[:, 2 * p:2 * p + 2,
                                              jc * 128:(jc + 1) * 128],
                                    rhs=p_t[:, 2 * p:2 * p + 2, isl],
                                    start=(p == 0), stop=(p == _CCH // 2 - 1),
                                    perf_mode=DR)
                        if jg >= 2:
                            av_mm(0, jg - 2, av01[0])
                            av_mm(1, jg - 2, av01[1])
                            if last and jg >= 10:
                                # last block: second denominator half summed
                                # by the PE (ones(x16) fp8 matmuls riding the
                                # es stream) -- the PE is idle at the end of
                                # the kernel while a DVE tree would put the
                                # whole tree+reciprocal chain on the tail
                                nc.tensor.matmul(
                                    rbc, lhsT=sb_ones8,
                                    rhs=es[:, 2 * (jg - 2):2 * (jg - 1), :],
                                    start=(jg == 10), stop=False,
                                    perf_mode=DR)
                        nc.scalar.activation(
                            out=es[:, jg * 2:(jg + 1) * 2, :].rearrange(
                                "p a b -> p (a b)"),
                            in_=ps.rearrange("p a b -> p (a b)"),
                            func=AF.Exp, scale=1.0 / _SP)
                        if jg % 4 == 3 and not (last and jg >= 8):
                            g = jg // 4
                            nc.vector.tensor_tensor(
                                out=l1[:, g * 4:(g + 1) * 4, :],
                                in0=es[:, 8 * g:8 * g + 4, :],
                                in1=es[:, 8 * g + 4:8 * (g + 1), :],
                                op=OP.add)
                        if jg == 7:
                            nc.vector.tensor_tensor(
                                out=l2[:, 0], in0=l1[:, 0:4, :],
                                in1=l1[:, 4:8, :], op=OP.add)
                            nc.vector.tensor_tensor(
                                out=l3[:, 0], in0=l2[:, 0, 0:2, :],
                                in1=l2[:, 0, 2:4, :], op=OP.add)
                            nc.vector.tensor_tensor(
                                out=dhalf[:, 0, :], in0=l3[:, 0, 0, :],
                                in1=l3[:, 0, 1, :], op=OP.add)
                        if jg == 15 and not last:
                            nc.vector.tensor_tensor(
                                out=l2[:, 1], in0=l1[:, 8:12, :],
                                in1=l1[:, 12:16, :], op=OP.add)
                            nc.vector.tensor_tensor(
                                out=l3[:, 1], in0=l2[:, 1, 0:2, :],
                                in1=l2[:, 1, 2:4, :], op=OP.add)
                            nc.vector.tensor_tensor(
                                out=dhalf[:, 1, :], in0=l3[:, 1, 0, :],
                                in1=l3[:, 1, 1, :], op=OP.add)
                    for jp in (14, 15):
                        av_mm(0, jp, av01[0])
                        av_mm(1, jp, av01[1])
                        if last:
                            nc.tensor.matmul(
                                rbc, lhsT=sb_ones8,
                                rhs=es[:, 2 * jp:2 * jp + 2, :],
                                start=False, stop=False, perf_mode=DR)
                    # O^T[c, i] = sum_j V'^T[j,c] expS^T[j,i]: this IS the
                    # (unnormalized) output -- no proj matmul afterwards.
                    # PSUM is evacuated by plain ACT copies so it never waits
                    # on the denominator; Pool/DVE finish the normalize.
                    # The rbc#2 matmul sits after AV(cc2) so the PE never
                    # waits for the trailing DVE half-tree.
                    rbc_sb = attw.tile([128, 512], f32, tag="rbc")
                    osbs = {}
                    for cc in (0, 1):
                        osb = resw.tile([128, 512], f32, tag="osb",
                                        name=f"osb{ib}_{cc}")
                        nc.scalar.copy(out=osb, in_=av01[cc])
                        osbs[cc] = osb
                    nc.tensor.matmul(rbc, lhsT=sb_obc16, rhs=dhalf[:, 0, :],
                                     start=not last, stop=last)
                    if last:
                        nc.vector.reciprocal(out=rbc_sb, in_=rbc)
                    pso2 = o_psum.tile([128, 512], f32, tag="o")
                    for jp in range(16):
                        av_mm(2, jp, pso2)
                    if not last:
                        nc.tensor.matmul(
                            rbc, lhsT=sb_obc16, rhs=dhalf[:, 1, :],
                            start=False, stop=True)
                        nc.vector.reciprocal(out=rbc_sb, in_=rbc)
                    attn_norm(osbs[0], rbc_sb, 0, isl, on_pool=not last)
                    attn_norm(osbs[1], rbc_sb, 1, isl, on_pool=False)
                    osb2 = resw.tile([128, 512], f32, tag="osb")
                    nc.scalar.copy(out=osb2, in_=pso2)
                    pso3 = o_psum.tile([128, 512], f32, tag="o")
                    for jp in range(16):
                        av_mm(3, jp, pso3)
                    attn_norm(osb2, rbc_sb, 2, isl, on_pool=not last)
                    osb3 = resw.tile([128, 512], f32, tag="osb")
                    nc.scalar.copy(out=osb3, in_=pso3)
                    attn_norm(osb3, rbc_sb, 3, isl, on_pool=False)

    _legalize_single_wait(nc, mybir)
    return nc


def kernel(**inputs):
    import ml_dtypes
    from concourse.bass_utils import run_bass_kernel_spmd

    global _cached
    if _cached is None:
        _cached = _build_program()
    nc = _cached

    x = np.asarray(inputs["x"], dtype=np.float32)
    gn_w = np.asarray(inputs["gn_w"], dtype=np.float32)
    gn_b = np.asarray(inputs["gn_b"], dtype=np.float32)
    wq = np.asarray(inputs["wq"], dtype=np.float32)
    bq = np.asarray(inputs["bq"], dtype=np.float32)
    wk = np.asarray(inputs["wk"], dtype=np.float32)
    wv = np.asarray(inputs["wv"], dtype=np.float32)
    bv = np.asarray(inputs["bv"], dtype=np.float32)
    wp = np.asarray(inputs["wp"], dtype=np.float32)
    bp = np.asarray(inputs["bp"], dtype=np.float32)
    # bk cancels in the j-softmax (it only adds per-query constants)

    fp8 = ml_dtypes.float8_e4m3
    scale = float(_C) ** -0.5

    def cols(v):  # [512] -> [128, 4] chunk columns
        return np.ascontiguousarray(v.reshape(_CCH, 128).T)

    def wlay(w, s):  # [cout, cin] -> lhsT chunked [128, cch*cout], fp8 x s
        return np.ascontiguousarray(
            w.T.reshape(_CCH, 128, _C).transpose(1, 0, 2).reshape(128, _CCH * _C)
            * s
        ).astype(fp8)

    wP = (wk.T.astype(np.float64) @ wq.astype(np.float64)).astype(np.float32)
    wV = (wp.astype(np.float64) @ wv.astype(np.float64)).astype(np.float32)
    cvec = wk.T @ bq

    consts = np.concatenate([
        cols(cvec * (_SA * scale)),                                 # bPa
        cols(cvec * (_SP * scale)),                                 # bPb
        cols(wp @ bv + bp),                                         # bpe2
        cols(gn_w),                                                 # gnw2
        cols(gn_b),                                                 # gnb2
        np.repeat(np.eye(8, dtype=np.float32), 16, axis=0) / 16.0,  # gmat
    ], axis=1)
    shared = {
        "wPT": wlay(wP, _SA * scale),
        "wVT": wlay(wV, _SV),
        "consts": consts,
        "obc16": np.full((128, 128), _SV2, ml_dtypes.bfloat16),
        "obc8": np.full((128, 256), _SV2, fp8),
        "gexp": np.repeat(np.eye(8, dtype=np.float32), 16, axis=1),
    }

    xf = x.reshape(_B, _C, _N)
    in_maps = []
    for core in range(_NCORES):
        bi, qh = core // 2, core % 2
        xbc = xf[bi]
        if qh == 1:  # rotate so this core's queries are columns 0..NQ-1
            xbc = np.concatenate([xbc[:, _NQ:], xbc[:, :_NQ]], axis=1)
        in_maps.append({
            "xb8": np.ascontiguousarray(xbc).astype(fp8),
            "xqf": np.ascontiguousarray(xbc[:, :_NQ], dtype=np.float32),
            **shared,
        })

    res = run_bass_kernel_spmd(nc, in_maps, core_ids=list(range(_NCORES)))

    out = np.empty((_B, _C, _N), np.float32)
    for core in range(_NCORES):
        bi, qh = core // 2, core % 2
        out[bi][:, qh * _NQ:(qh + 1) * _NQ] = res.results[core]["out"]
    return out.reshape(_B, _C, 64, 64)


# revision 33
# speedup vs baseline: 1.3846x; 1.0228x over previous
"""AttnBlock (GroupNorm -> qkv 1x1 -> NxN spatial attention -> proj -> residual)
for Trainium2, SPMD over 8 NeuronCores.

Sharding: core = (batch b in 0..3, query-half qh in 0..1): the host rotates
the spatial columns so each core's 2048 queries are columns 0..2047 of its
input; keys are the full 4096 columns (key order is irrelevant to softmax
attention, and only query columns are written back).

Algebraic restructure vs the straightforward q/k/v/proj pipeline (exact, no
approximation):
  * scores_ij = (Wq hn_i + bq).(Wk hn_j + bk); per-query terms cancel in the
    j-softmax, so scores ~ P_i . hn_j with P = scale*((Wk^T Wq) hn + Wk^T bq).
    The K projection disappears: keys are raw hn, queries get one projection
    by the host-precomputed 512x512 matrix Wk^T Wq.
  * attention rows sum to 1, so the output projection commutes through the
    value sum: out_i = sum_j a_ij (Wp Wv hn)_j + (Wp bv + bp).  V is projected
    once by the host-precomputed Wp Wv; the separate proj matmul disappears.
Both precomputed matrices are weight-only (input-independent), like the bias
folds.  This removes ~25% of the tensor-engine work.

Engine budget per 512-query attention block (PE is the roofline at ~27.5us):
PE S^T 13.7 + AV 13.7; ACT exp 17.8 + psum evacuation 2.7; DVE denominator
tree ~8 + reciprocal 3.4; Pool normalize/bias/residual chain ~16 + out DMA.
The AV psum is evacuated by plain ACT copies so no PSUM buffer ever waits on
the softmax denominator (which needs the full tree + reciprocal); the
denominator is reduced in two halves with the ones-matmul accumulating both
into one PSUM bank, so only half a tree remains after the last exp.

GroupNorm: x is loaded once as fp8 (stats and normalize both read it; the
fp32 residual copy streams in during the attention phase).  Chunk 0 uses
classic sum/sumsq on ACT, chunks 1-3 single-pass BN_STATS on DVE, per-chunk
group chains run interleaved with the later chunks' stats, normalizes split
ACT/DVE.  Engines execute in program order, so ops are emitted in data-arrival
order; the x DMAs are first on both trigger queues.
"""

import numpy as np

_B, _C, _HW = 4, 512, 64 * 64  # batch, channels, spatial N
_N = _HW                       # 4096
_NQ = _N // 2                  # queries per core
_G = 32                        # groupnorm groups
_EPS = 1e-6
_NCORES = 8
_CCH = _C // 128               # 4 channel chunks

# fp8 range scaling (host pre-mult, device descale; all powers of two)
_SA = 4096.0   # on Wk^T Wq entries (~4e-4 std)
_SP = 256.0    # on P in SBUF (~0.01 std)
_SV = 512.0    # on Wp Wv entries (~0.009 std)
_SV2 = 16.0    # on V' in SBUF (~0.2 std); folded back via the ones-matrix

_cached = None  # (nc,) built Bass program, reused across kernel() calls


def _legalize_single_wait(nc, mybir):
    """This container's walrus codegen accepts at most ONE sync-wait per
    instruction. Tile emits N-wait instructions; hoist the extras onto
    injected same-engine NOPs placed immediately before."""
    ctr = 0
    for f in nc.m.functions:
        for bb in f.blocks:
            out = []
            changed = False
            for inst in bb.instructions:
                si = inst.sync_info
                if si is not None and len(si.on_wait) > 1:
                    waits = list(si.on_wait)
                    for w in waits[:-1]:
                        ctr += 1
                        out.append(mybir.InstNoOp(
                            name=f"I-legalize-wait-{ctr}",
                            engine=inst.engine,
                            sync_info=mybir.SyncInfo(on_wait=[w], on_update=[]),
                        ))
                    inst.sync_info = mybir.SyncInfo(
                        on_wait=[waits[-1]], on_update=list(si.on_update))
                    changed = True
                out.append(inst)
            if changed:
                bb.instructions = out


def _build_program():
    import concourse.bass as bass
    import concourse.tile as tile
    import concourse.mybir as mybir

    f32 = mybir.dt.float32
    bf16 = mybir.dt.bfloat16
    fp8 = mybir.dt.float8e4
    DR = mybir.MatmulPerfMode.DoubleRow
    AF = mybir.ActivationFunctionType
    OP = mybir.AluOpType

    nc = bass.Bass(name="attnblock")

    xb8 = nc.declare_dram_parameter("xb8", [_C, _N], fp8, isOutput=False)
    xqf = nc.declare_dram_parameter("xqf", [_C, _NQ], f32, isOutput=False)
    wPT = nc.declare_dram_parameter("wPT", [128, _CCH * _C], fp8, isOutput=False)
    wVT = nc.declare_dram_parameter("wVT", [128, _CCH * _C], fp8, isOutput=False)
    # all small [128, x] f32 constants packed into one tensor:
    # [bPa(4) | bPb(4) | bpe2(4) | gnw2(4) | gnb2(4) | gmat(8)]
    consts = nc.declare_dram_parameter("consts", [128, 28], f32, isOutput=False)
    obc16 = nc.declare_dram_parameter("obc16", [128, 128], bf16, isOutput=False)
    obc8 = nc.declare_dram_parameter("obc8", [128, 256], fp8, isOutput=False)
    gexp = nc.declare_dram_parameter("gexp", [8, 128], f32, isOutput=False)
    out_d = nc.declare_dram_parameter("out", [_C, _NQ], f32, isOutput=True)

    QW = _N // 4    # 1024: classic-stat quarter
    HW2 = _N // 2

    with tile.TileContext(nc) as tc:
        with (
            tc.tile_pool(name="singles", bufs=1) as singles,
            tc.tile_pool(name="persist", bufs=1) as persist,
        ):
            # ---- input DMAs first on both trigger queues -----------------
            # x (fp8) chunks as SEPARATE half-tiles: DMA-written tiles are
            # waited at tile granularity, so a single whole-chunk tile would
            # stall the first stats op until both halves' queues drained.
            dma_engs = [nc.sync, nc.gpsimd]
            xth = []
            k = 0
            for ci in range(_CCH):
                halves = []
                for h in range(2):
                    xt = persist.tile([128, HW2], fp8, tag=f"xt{ci}_{h}",
                                      name=f"xt{ci}_{h}")
                    eng = dma_engs[k % 2]
                    k += 1
                    sl = slice(h * HW2, (h + 1) * HW2)
                    eng.dma_start(out=xt,
                                  in_=xb8[ci * 128:(ci + 1) * 128, sl])
                    halves.append(xt)
                xth.append(halves)

            # ---- constants / weights -------------------------------------
            sb_consts = singles.tile([128, 28], f32, tag="consts")
            nc.sync.dma_start(out=sb_consts, in_=consts[:, :])
            sb_bPa = sb_consts[:, 0:4]    # S_A*scale*(Wk^T bq)   (DVE copies)
            sb_bPb = sb_consts[:, 4:8]    # S_P*scale*(Wk^T bq)   (ACT copies)
            sb_bpe = sb_consts[:, 8:12]   # Wp bv + bp
            sb_gnw = sb_consts[:, 12:16]
            sb_gnb = sb_consts[:, 16:20]
            sb_gmat = sb_consts[:, 20:28]  # eye-repeat(8)/16
            sb_gexp = singles.tile([8, 128], f32, tag="gexp")
            nc.sync.dma_start(out=sb_gexp, in_=gexp[:, :])
            sb_obc16 = singles.tile([128, 128], bf16, tag="obc16")
            nc.sync.dma_start(out=sb_obc16, in_=obc16[:, :])
            sb_ones8 = singles.tile([128, 2, 128], fp8, tag="ones8")
            nc.sync.dma_start(
                out=sb_ones8, in_=obc8.rearrange("p (a f) -> p a f", a=2))
            # the two precombined weight matrices (needed ~30us in)
            w_tiles = {}
            for nm, src, eng in (("wP", wPT, nc.sync),
                                 ("wV", wVT, nc.gpsimd)):
                t = singles.tile([128, _CCH, _C], fp8, tag=f"w_{nm}",
                                 name=f"w_{nm}")
                eng.dma_start(
                    out=t, in_=src.rearrange("p (a f) -> p a f", a=_CCH))
                w_tiles[nm] = t

            sb_eps8 = singles.tile([8, 1], f32, tag="eps8")
            nc.vector.memset(sb_eps8, _EPS)
            sb_warm = singles.tile([128, 1], f32, tag="warm1")
            nc.vector.memset(sb_warm, 1.0)
            # ACT table prep: GN needs Sqrt/Square/Identity, all served by
            # the sqrt_and_others table.  Exp is touched once after the last
            # GN ACT op so the exp table load lands during the projections.
            sb_actw = singles.tile([8, 2], f32, tag="actw")
            nc.scalar.activation(out=sb_actw[:, 0:1], in_=sb_eps8, func=AF.Sqrt)
            nc.scalar.activation(out=sb_actw[:, 1:2], in_=sb_eps8,
                                 func=AF.Square)

            # normalize constants per chunk: mu', rstd', -mu'*rstd'
            musig = singles.tile([128, _CCH, 3], f32, tag="musig")

            # hn (normalized x, fp8) packed [c_lo, chunk, N]
            hn_t = persist.tile([128, _CCH, _N], fp8, tag="hn")

            # ---- phase 1: GroupNorm --------------------------------------
            with (
                tc.tile_pool(name="gn_small", bufs=2) as gn_small,
                tc.tile_pool(name="gn_psum", bufs=2, space="PSUM") as gn_psum,
                tc.tile_pool(name="warm_psum", bufs=1, space="PSUM") as warm_psum,
            ):
                # PE warm-up: keep the tensor engine busy through the GN head
                # so the HAM clock is ramped when the projections start.
                warm_ps = warm_psum.tile([128, 512], f32, tag="warm")

                def warm(n_small, n_big):
                    for _ in range(n_small):
                        nc.tensor.matmul(warm_ps[0:1, 0:1], lhsT=sb_warm,
                                         rhs=sb_warm, start=True, stop=True)
                    for _ in range(n_big):
                        nc.tensor.matmul(warm_ps, lhsT=xth[0][0][:, 0:128],
                                         rhs=xth[0][0][:, 0:512],
                                         start=True, stop=True)

                warm(80, 12)

                # ALL per-chunk stats first, then all chains: the chains hop
                # engines (PE/ACT round trips), and anything emitted between
                # two chunks' stats stalls the DVE queue for multiple us.
                # Chunks 0/1: classic sum/sumsq on ACT (idle early); chunks
                # 2/3: single-pass BN_STATS on DVE; work rebalanced so both
                # engines run ~10us of stats and finish together.
                t2s = [gn_small.tile([128, 2], f32, tag=f"t2_{ci}",
                                     name=f"t2_{ci}") for ci in range(_CCH)]
                sparts = {}
                for ci in (0, 1):  # ACT side (accumulators only; reduces on
                    spart = gn_small.tile([128, 4], f32, tag=f"sp_{ci}",
                                          name=f"sp_{ci}")
                    qpart = gn_small.tile([128, 4], f32, tag=f"qp_{ci}",
                                          name=f"qp_{ci}")
                    sparts[ci] = (spart, qpart)
                    for h in range(4):
                        src = xth[ci][h // 2][:, (h % 2) * QW:
                                              (h % 2 + 1) * QW]
                        qs = slice(h * QW, (h + 1) * QW)
                        nc.scalar.activation(out=hn_t[:, ci, qs],
                                             in_=src,
                                             func=AF.Square,
                                             accum_out=qpart[:, h:h + 1])
                        nc.scalar.activation(out=hn_t[:, ci, qs],
                                             in_=src,
                                             func=AF.Identity,
                                             accum_out=spart[:, h:h + 1])
                    warm(0, 7)
                for ci in (2, 3):  # DVE side
                    t2 = t2s[ci]
                    bn8 = gn_small.tile([128, 8, 6], f32, tag=f"bn8_{ci}",
                                        name=f"bn8_{ci}")
                    for s in range(8):
                        nc.vector.bn_stats(
                            out=bn8[:, s, :],
                            in_=xth[ci][s // 4][:, (s % 4) * 512:
                                                (s % 4 + 1) * 512])
                    mv = gn_small.tile([128, 2], f32, tag=f"mv_{ci}",
                                       name=f"mv_{ci}")
                    nc.vector.bn_aggr(out=mv, in_=bn8)
                    sqm = gn_small.tile([128, 1], f32, tag="sqm")
                    nc.vector.tensor_mul(sqm, mv[:, 0:1], mv[:, 0:1])
                    nc.vector.tensor_copy(out=t2[:, 0:1], in_=mv[:, 0:1])
                    nc.vector.tensor_tensor(
                        out=t2[:, 1:2], in0=mv[:, 1:2], in1=sqm,
                        op=OP.add)
                    warm(0, 7)
                for ci in (0, 1):  # classic reduces (after the bn queue)
                    t2 = t2s[ci]
                    spart, qpart = sparts[ci]
                    nc.vector.reduce_sum(out=t2[:, 0:1], in_=spart,
                                         axis=mybir.AxisListType.XYZW)
                    nc.vector.reduce_sum(out=t2[:, 1:2], in_=qpart,
                                         axis=mybir.AxisListType.XYZW)
                    nc.vector.tensor_scalar_mul(
                        out=t2[:, 0:1], in0=t2[:, 0:1], scalar1=1.0 / _N)
                    nc.vector.tensor_scalar_mul(
                        out=t2[:, 1:2], in0=t2[:, 1:2], scalar1=1.0 / _N)

                for ci in range(_CCH):
                    # group chain (PSUM copies on DVE so the ACT queue stays
                    # free for the normalizes)
                    pg = gn_psum.tile([8, 2], f32, tag="pg")
                    nc.tensor.matmul(pg, lhsT=sb_gmat, rhs=t2s[ci],
                                     start=True, stop=True)
                    gs = gn_small.tile([8, 2], f32, tag="gs")
                    nc.vector.tensor_copy(out=gs, in_=pg)
                    # var_g = m2 - mu^2 ; rstd_g = 1/sqrt(var+eps)
                    musq = gn_small.tile([8, 1], f32, tag="musq")
                    nc.vector.tensor_mul(musq, gs[:, 0:1], gs[:, 0:1])
                    nc.vector.tensor_tensor(
                        out=gs[:, 1:2], in0=gs[:, 1:2], in1=musq,
                        op=OP.subtract)
                    sq8 = gn_small.tile([8, 1], f32, tag="sq8")
                    nc.scalar.activation(
                        out=sq8, in_=gs[:, 1:2], func=AF.Sqrt, bias=sb_eps8)
                    nc.vector.reciprocal(out=gs[:, 1:2], in_=sq8)
                    # broadcast to channels: [128, 2] = gexp.T @ [mu_g, rstd_g]
                    pc = gn_psum.tile([128, 2], f32, tag="pc")
                    nc.tensor.matmul(pc, lhsT=sb_gexp, rhs=gs, start=True,
                                     stop=True)
                    pcs = gn_small.tile([128, 2], f32, tag="pcs")
                    nc.vector.tensor_copy(out=pcs, in_=pc)
                    # fold gamma/beta: rstd' = rstd*gamma; mu' = mu - beta/rstd'
                    nc.vector.tensor_mul(
                        musig[:, ci, 1:2], pcs[:, 1:2], sb_gnw[:, ci:ci + 1])
                    rec = gn_small.tile([128, 1], f32, tag="rec")
                    nc.vector.reciprocal(out=rec, in_=musig[:, ci, 1:2])
                    bs = gn_small.tile([128, 1], f32, tag="bs")
                    nc.vector.tensor_mul(bs, sb_gnb[:, ci:ci + 1], rec)
                    nc.vector.tensor_tensor(
                        out=musig[:, ci, 0:1], in0=pcs[:, 0:1], in1=bs,
                        op=OP.subtract)
                    nc.vector.scalar_tensor_tensor(
                        out=musig[:, ci, 2:3], in0=musig[:, ci, 0:1],
                        scalar=-1.0, in1=musig[:, ci, 1:2],
                        op0=OP.mult, op1=OP.mult)
                    # hn = (x - mu') * rstd' (fp8), 5 halves on DVE (1.2us
                    # each) and 3 on ACT (1.9us each) so both finish together
                    for h in range(2):
                        hsl = slice(h * HW2, (h + 1) * HW2)
                        if ci < 2 or (ci == 2 and h == 0):
                            nc.vector.tensor_scalar(
                                out=hn_t[:, ci, hsl], in0=xth[ci][h],
                                scalar1=musig[:, ci, 0:1],
                                scalar2=musig[:, ci, 1:2],
                                op0=OP.subtract, op1=OP.mult)
                        else:
                            nc.scalar.activation(
                                out=hn_t[:, ci, hsl], in_=xth[ci][h],
                                func=AF.Identity, scale=musig[:, ci, 1:2],
                                bias=musig[:, ci, 2:3])
                    warm(0, 4)
                # preload the exp table while the projections run
                nc.scalar.activation(out=sb_actw[:, 0:1], in_=sb_eps8,
                                     func=AF.Exp)
                warm(0, 8)

            # ---- phase 2: P and V'^T projections -------------------------
            p_t = persist.tile([128, _CCH, _NQ], fp8, tag="P")
            vt_t = persist.tile([128, 32, _C], fp8, tag="VT")

            with (
                tc.tile_pool(name="kq_psum", bufs=2, space="PSUM") as kq_psum,
                tc.tile_pool(name="vt_psum", bufs=2, space="PSUM") as vt_psum,
            ):
                # P[o]: queries only; copies alternate DVE/ACT
                for o in range(_CCH):
                    osl = slice(o * 128, (o + 1) * 128)
                    for jg in range(_NQ // 1024):
                        ps = kq_psum.tile([128, 2, 512], f32, tag="kq")
                        for jj in range(2):
                            j0 = jg * 1024 + jj * 512
                            for p in range(_CCH // 2):
                                nc.tensor.matmul(
                                    ps[:, jj, :],
                                    lhsT=w_tiles["wP"][:, 2 * p:2 * p + 2, osl],
                                    rhs=hn_t[:, 2 * p:2 * p + 2, j0:j0 + 512],
                                    start=(p == 0), stop=(p == _CCH // 2 - 1),
                                    perf_mode=DR)
                        dst = p_t[:, o, jg * 1024:(jg + 1) * 1024]
                        src = ps.rearrange("p a b -> p (a b)")
                        if jg % 2 == 0:
                            nc.vector.tensor_scalar(
                                out=dst, in0=src,
                                scalar1=sb_bPa[:, o:o + 1], scalar2=_SP / _SA,
                                op0=OP.add, op1=OP.mult)
                        else:
                            nc.scalar.activation(
                                out=dst, in_=src, func=AF.Identity,
                                scale=_SP / _SA, bias=sb_bPb[:, o:o + 1])
                # V'^T[j, c]: stationary = hn column slices; two j-tiles per
                # PSUM tile so each evacuation copy moves 1024 columns (the
                # copies, not the matmuls, are the V' throughput limit)
                for jp in range(16):
                    ps2 = vt_psum.tile([128, 2, 512], f32, tag="vt")
                    for jj in range(2):
                        jc = 2 * jp + jj
                        for p in range(_CCH // 2):
                            nc.tensor.matmul(
                                ps2[:, jj, :],
                                lhsT=hn_t[:, 2 * p:2 * p + 2,
                                          jc * 128:(jc + 1) * 128],
                                rhs=w_tiles["wV"][:, 2 * p:2 * p + 2, :],
                                start=(p == 0), stop=(p == _CCH // 2 - 1),
                                perf_mode=DR)
                    dst = vt_t[:, 2 * jp:2 * jp + 2, :].rearrange(
                        "p a b -> p (a b)")
                    src = ps2.rearrange("p a b -> p (a b)")
                    if jp % 2 == 0:
                        nc.scalar.mul(out=dst, in_=src, mul=_SV2 / _SV)
                    else:
                        nc.vector.tensor_scalar_mul(
                            out=dst, in0=src, scalar1=_SV2 / _SV)

            # ---- phase 3: attention + residual, per 512-query block ------
            with (
                tc.tile_pool(name="attw", bufs=1) as attw,
                tc.tile_pool(name="resw", bufs=4) as resw,
                tc.tile_pool(name="s_psum", bufs=2, space="PSUM") as s_psum,
                tc.tile_pool(name="o_psum", bufs=2, space="PSUM") as o_psum,
                tc.tile_pool(name="pd_psum", bufs=2, space="PSUM") as pd_psum,
            ):
                def attn_norm(osb, rbc_sb, cc, isl, on_pool):
                    """normalize by 1/denom, + (Wp bv + bp) + residual, out.
                    Half the chains ride the otherwise-idle Pool engine; the
                    rest use DVE's fused scalar_tensor_tensor (also: all of
                    the last block's, since Pool ops run ~1.3us each and
                    would stretch the kernel's tail)."""
                    xres = resw.tile([128, 512], f32, tag="xres")
                    nc.sync.dma_start(
                        out=xres, in_=xqf[cc * 128:(cc + 1) * 128, isl])
                    outt = resw.tile([128, 512], f32, tag="outt")
                    if on_pool:
                        t1 = resw.tile([128, 512], f32, tag="t1")
                        nc.gpsimd.tensor_tensor(
                            out=t1, in0=osb, in1=rbc_sb, op=OP.mult)
                        o1 = resw.tile([128, 512], f32, tag="o1")
                        nc.gpsimd.tensor_scalar(
                            out=o1, in0=t1, scalar1=sb_bpe[:, cc:cc + 1],
                            scalar2=1.0, op0=OP.add, op1=OP.mult)
                        nc.gpsimd.tensor_tensor(
                            out=outt, in0=o1, in1=xres, op=OP.add)
                        nc.gpsimd.dma_start(
                            out=out_d[cc * 128:(cc + 1) * 128, isl], in_=outt)
                    else:
                        t1 = resw.tile([128, 512], f32, tag="t1")
                        nc.vector.tensor_tensor(
                            out=t1, in0=osb, in1=rbc_sb, op=OP.mult)
                        nc.vector.scalar_tensor_tensor(
                            out=outt, in0=t1, scalar=sb_bpe[:, cc:cc + 1],
                            in1=xres, op0=OP.add, op1=OP.add)
                        nc.sync.dma_start(
                            out=out_d[cc * 128:(cc + 1) * 128, isl], in_=outt)

                for ib in range(_NQ // 512):
                    last = ib == (_NQ // 512) - 1
                    isl = slice(ib * 512, (ib + 1) * 512)
                    es = attw.tile([128, 32, 512], fp8, tag="ES", bufs=2)
                    l1 = attw.tile([128, 16, 512], bf16, tag="L1")
                    l2 = attw.tile([128, 2, 4, 512], bf16, tag="L2")
                    l3 = attw.tile([128, 2, 2, 512], bf16, tag="L3")
                    dhalf = attw.tile([128, 2, 512], bf16, tag="dh")
                    rbc = pd_psum.tile([128, 512], f32, tag="pd")
                    # The S^T matmuls alone outrun the exps (PSUM allows only
                    # 2 tiles in flight, so the PE would stall at the exps'
                    # pace); interleaving the AV accumulation for channel
                    # chunks 0/1 (whose es inputs are 2 groups behind) keeps
                    # the PE fed at exactly the rate ACT can sustain.
                    # Denominator add-tree in two halves (contiguous reads);
                    # the ones(x16)-matmul accumulates both halves into one
                    # PSUM bank and broadcasts the j-total to all partitions.
                    av01 = [o_psum.tile([128, 512], f32, tag="o",
                                        name=f"av{ib}_{c}")
                            for c in range(2)]

                    def av_mm(cc, jp, pso):
                        nc.tensor.matmul(
                            pso,
                            lhsT=vt_t[:, 2 * jp:2 * jp + 2,
                                      cc * 128:(cc + 1) * 128],
                            rhs=es[:, 2 * jp:2 * jp + 2, :],
                            start=(jp == 0), stop=(jp == 15),
                            perf_mode=DR)

                    for jg in range(16):
                        ps = s_psum.tile([128, 2, 512], f32, tag="s")
                        for jj in range(2):
                            jc = jg * 2 + jj
                            for p in range(_CCH // 2):
                                nc.tensor.matmul(
                                    ps[:, jj, :],
                                    lhsT=hn_t[:, 2 * p:2 * p + 2,
                                              jc * 128:(jc + 1) * 128],
                                    rhs=p_t[:, 2 * p:2 * p + 2, isl],
                                    start=(p == 0), stop=(p == _CCH // 2 - 1),
                                    perf_mode=DR)
                        if jg >= 2:
                            av_mm(0, jg - 2, av01[0])
                            av_mm(1, jg - 2, av01[1])
                            if jg >= 10:
                                # second denominator half summed by the PE
                                # (ones(x16) fp8 matmuls riding the es
                                # stream): 1.7us of PE buys back ~7us of the
                                # DVE add-tree, whose fp8 reads run at only
                                # ~1.3ns/col
                                nc.tensor.matmul(
                                    rbc, lhsT=sb_ones8,
                                    rhs=es[:, 2 * (jg - 2):2 * (jg - 1), :],
                                    start=(jg == 10), stop=False,
                                    perf_mode=DR)
                        nc.scalar.activation(
                            out=es[:, jg * 2:(jg + 1) * 2, :].rearrange(
                                "p a b -> p (a b)"),
                            in_=ps.rearrange("p a b -> p (a b)"),
                            func=AF.Exp, scale=1.0 / _SP)
                        if jg % 4 == 3 and jg < 8:
                            g = jg // 4
                            nc.vector.tensor_tensor(
                                out=l1[:, g * 4:(g + 1) * 4, :],
                                in0=es[:, 8 * g:8 * g + 4, :],
                                in1=es[:, 8 * g + 4:8 * (g + 1), :],
                                op=OP.add)
                        if jg == 7:
                            nc.vector.tensor_tensor(
                                out=l2[:, 0], in0=l1[:, 0:4, :],
                                in1=l1[:, 4:8, :], op=OP.add)
                            nc.vector.tensor_tensor(
                                out=l3[:, 0], in0=l2[:, 0, 0:2, :],
                                in1=l2[:, 0, 2:4, :], op=OP.add)
                            nc.vector.tensor_tensor(
                                out=dhalf[:, 0, :], in0=l3[:, 0, 0, :],
                                in1=l3[:, 0, 1, :], op=OP.add)
                    for jp in (14, 15):
                        av_mm(0, jp, av01[0])
                        av_mm(1, jp, av01[1])
                        nc.tensor.matmul(
                            rbc, lhsT=sb_ones8,
                            rhs=es[:, 2 * jp:2 * jp + 2, :],
                            start=False, stop=False, perf_mode=DR)
                    # O^T[c, i] = sum_j V'^T[j,c] expS^T[j,i]: this IS the
                    # (unnormalized) output -- no proj matmul afterwards.
                    # PSUM is evacuated by plain ACT copies so it never waits
                    # on the denominator; Pool/DVE finish the normalize.
                    # The rbc#2 matmul sits after AV(cc2) so the PE never
                    # waits for the trailing DVE half-tree.
                    rbc_sb = attw.tile([128, 512], f32, tag="rbc")
                    osbs = {}
                    for cc in (0, 1):
                        osb = resw.tile([128, 512], f32, tag="osb",
                                        name=f"osb{ib}_{cc}")
                        nc.scalar.copy(out=osb, in_=av01[cc])
                        osbs[cc] = osb
                    nc.tensor.matmul(rbc, lhsT=sb_obc16, rhs=dhalf[:, 0, :],
                                     start=False, stop=True)
                    nc.vector.reciprocal(out=rbc_sb, in_=rbc)
                    pso2 = o_psum.tile([128, 512], f32, tag="o")
                    for jp in range(16):
                        av_mm(2, jp, pso2)
                    attn_norm(osbs[0], rbc_sb, 0, isl, on_pool=not last)
                    attn_norm(osbs[1], rbc_sb, 1, isl, on_pool=False)
                    osb2 = resw.tile([128, 512], f32, tag="osb")
                    nc.scalar.copy(out=osb2, in_=pso2)
                    pso3 = o_psum.tile([128, 512], f32, tag="o")
                    for jp in range(16):
                        av_mm(3, jp, pso3)
                    attn_norm(osb2, rbc_sb, 2, isl, on_pool=not last)
                    osb3 = resw.tile([128, 512], f32, tag="osb")
                    nc.scalar.copy(out=osb3, in_=pso3)
                    attn_norm(osb3, rbc_sb, 3, isl, on_pool=False)

    _legalize_single_wait(nc, mybir)
    return nc


def kernel(**inputs):
    import ml_dtypes
    from concourse.bass_utils import run_bass_kernel_spmd

    global _cached
    if _cached is None:
        _cached = _build_program()
    nc = _cached

    x = np.asarray(inputs["x"], dtype=np.float32)
    gn_w = np.asarray(inputs["gn_w"], dtype=np.float32)
    gn_b = np.asarray(inputs["gn_b"], dtype=np.float32)
    wq = np.asarray(inputs["wq"], dtype=np.float32)
    bq = np.asarray(inputs["bq"], dtype=np.float32)
    wk = np.asarray(inputs["wk"], dtype=np.float32)
    wv = np.asarray(inputs["wv"], dtype=np.float32)
    bv = np.asarray(inputs["bv"], dtype=np.float32)
    wp = np.asarray(inputs["wp"], dtype=np.float32)
    bp = np.asarray(inputs["bp"], dtype=np.float32)
    # bk cancels in the j-softmax (it only adds per-query constants)

    fp8 = ml_dtypes.float8_e4m3
    scale = float(_C) ** -0.5

    def cols(v):  # [512] -> [128, 4] chunk columns
        return np.ascontiguousarray(v.reshape(_CCH, 128).T)

    def wlay(w, s):  # [cout, cin] -> lhsT chunked [128, cch*cout], fp8 x s
        return np.ascontiguousarray(
            w.T.reshape(_CCH, 128, _C).transpose(1, 0, 2).reshape(128, _CCH * _C)
            * s
        ).astype(fp8)

    wP = (wk.T.astype(np.float64) @ wq.astype(np.float64)).astype(np.float32)
    wV = (wp.astype(np.float64) @ wv.astype(np.float64)).astype(np.float32)
    cvec = wk.T @ bq

    consts = np.concatenate([
        cols(cvec * (_SA * scale)),                                 # bPa
        cols(cvec * (_SP * scale)),                                 # bPb
        cols(wp @ bv + bp),                                         # bpe2
        cols(gn_w),                                                 # gnw2
        cols(gn_b),                                                 # gnb2
        np.repeat(np.eye(8, dtype=np.float32), 16, axis=0) / 16.0,  # gmat
    ], axis=1)
    shared = {
        "wPT": wlay(wP, _SA * scale),
        "wVT": wlay(wV, _SV),
        "consts": consts,
        "obc16": np.full((128, 128), _SV2, ml_dtypes.bfloat16),
        "obc8": np.full((128, 256), _SV2, fp8),
        "gexp": np.repeat(np.eye(8, dtype=np.float32), 16, axis=1),
    }

    xf = x.reshape(_B, _C, _N)
    in_maps = []
    for core in range(_NCORES):
        bi, qh = core // 2, core % 2
        xbc = xf[bi]
        if qh == 1:  # rotate so this core's queries are columns 0..NQ-1
            xbc = np.concatenate([xbc[:, _NQ:], xbc[:, :_NQ]], axis=1)
        in_maps.append({
            "xb8": np.ascontiguousarray(xbc).astype(fp8),
            "xqf": np.ascontiguousarray(xbc[:, :_NQ], dtype=np.float32),
            **shared,
        })

    res = run_bass_kernel_spmd(nc, in_maps, core_ids=list(range(_NCORES)))

    out = np.empty((_B, _C, _N), np.float32)
    for core in range(_NCORES):
        bi, qh = core // 2, core % 2
        out[bi][:, qh * _NQ:(qh + 1) * _NQ] = res.results[core]["out"]
    return out.reshape(_B, _C, 64, 64)


# revision 35
# speedup vs baseline: 1.3976x; 1.0094x over previous
"""AttnBlock (GroupNorm -> qkv 1x1 -> NxN spatial attention -> proj -> residual)
for Trainium2, SPMD over 8 NeuronCores.

Sharding: core = (batch b in 0..3, query-half qh in 0..1): the host rotates
the spatial columns so each core's 2048 queries are columns 0..2047 of its
input; keys are the full 4096 columns (key order is irrelevant to softmax
attention, and only query columns are written back).

Algebraic restructure vs the straightforward q/k/v/proj pipeline (exact, no
approximation):
  * scores_ij = (Wq hn_i + bq).(Wk hn_j + bk); per-query terms cancel in the
    j-softmax, so scores ~ P_i . hn_j with P = scale*((Wk^T Wq) hn + Wk^T bq).
    The K projection disappears: keys are raw hn, queries get one projection
    by the host-precomputed 512x512 matrix Wk^T Wq.
  * attention rows sum to 1, so the output projection commutes through the
    value sum: out_i = sum_j a_ij (Wp Wv hn)_j + (Wp bv + bp).  V is projected
    once by the host-precomputed Wp Wv; the separate proj matmul disappears.
Both precomputed matrices are weight-only (input-independent), like the bias
folds.  This removes ~25% of the tensor-engine work.

Engine budget per 512-query attention block (PE is the roofline at ~27.5us):
PE S^T 13.7 + AV 13.7; ACT exp 17.8 + psum evacuation 2.7; DVE denominator
tree ~8 + reciprocal 3.4; Pool normalize/bias/residual chain ~16 + out DMA.
The AV psum is evacuated by plain ACT copies so no PSUM buffer ever waits on
the softmax denominator (which needs the full tree + reciprocal); the
denominator is reduced in two halves with the ones-matmul accumulating both
into one PSUM bank, so only half a tree remains after the last exp.

GroupNorm: x is loaded once as fp8 (stats and normalize both read it; the
fp32 residual copy streams in during the attention phase).  Chunk 0 uses
classic sum/sumsq on ACT, chunks 1-3 single-pass BN_STATS on DVE, per-chunk
group chains run interleaved with the later chunks' stats, normalizes split
ACT/DVE.  Engines execute in program order, so ops are emitted in data-arrival
order; the x DMAs are first on both trigger queues.
"""

import numpy as np

_B, _C, _HW = 4, 512, 64 * 64  # batch, channels, spatial N
_N = _HW                       # 4096
_NQ = _N // 2                  # queries per core
_G = 32                        # groupnorm groups
_EPS = 1e-6
_NCORES = 8
_CCH = _C // 128               # 4 channel chunks

# fp8 range scaling (host pre-mult, device descale; all powers of two)
_SA = 4096.0   # on Wk^T Wq entries (~4e-4 std)
_SP = 256.0    # on P in SBUF (~0.01 std)
_SV = 512.0    # on Wp Wv entries (~0.009 std)
_SV2 = 16.0    # on V' in SBUF (~0.2 std); folded back via the ones-matrix

_cached = None  # (nc,) built Bass program, reused across kernel() calls


def _legalize_single_wait(nc, mybir):
    """This container's walrus codegen accepts at most ONE sync-wait per
    instruction. Tile emits N-wait instructions; hoist the extras onto
    injected same-engine NOPs placed immediately before."""
    ctr = 0
    for f in nc.m.functions:
        for bb in f.blocks:
            out = []
            changed = False
            for inst in bb.instructions:
                si = inst.sync_info
                if si is not None and len(si.on_wait) > 1:
                    waits = list(si.on_wait)
                    for w in waits[:-1]:
                        ctr += 1
                        out.append(mybir.InstNoOp(
                            name=f"I-legalize-wait-{ctr}",
                            engine=inst.engine,
                            sync_info=mybir.SyncInfo(on_wait=[w], on_update=[]),
                        ))
                    inst.sync_info = mybir.SyncInfo(
                        on_wait=[waits[-1]], on_update=list(si.on_update))
                    changed = True
                out.append(inst)
            if changed:
                bb.instructions = out


def _build_program():
    import concourse.bass as bass
    import concourse.tile as tile
    import concourse.mybir as mybir

    f32 = mybir.dt.float32
    bf16 = mybir.dt.bfloat16
    fp8 = mybir.dt.float8e4
    DR = mybir.MatmulPerfMode.DoubleRow
    AF = mybir.ActivationFunctionType
    OP = mybir.AluOpType

    nc = bass.Bass(name="attnblock")

    xb8 = nc.declare_dram_parameter("xb8", [_C, _N], fp8, isOutput=False)
    xqf = nc.declare_dram_parameter("xqf", [_C, _NQ], f32, isOutput=False)
    wPT = nc.declare_dram_parameter("wPT", [128, _CCH * _C], fp8, isOutput=False)
    wVT = nc.declare_dram_parameter("wVT", [128, _CCH * _C], fp8, isOutput=False)
    # all small [128, x] f32 constants packed into one tensor:
    # [bPa(4) | bPb(4) | bpe2(4) | gnw2(4) | gnb2(4) | gmat(8)]
    consts = nc.declare_dram_parameter("consts", [128, 28], f32, isOutput=False)
    obc16 = nc.declare_dram_parameter("obc16", [128, 128], bf16, isOutput=False)
    obc8 = nc.declare_dram_parameter("obc8", [128, 256], fp8, isOutput=False)
    gexp = nc.declare_dram_parameter("gexp", [8, 128], f32, isOutput=False)
    out_d = nc.declare_dram_parameter("out", [_C, _NQ], f32, isOutput=True)

    QW = _N // 4    # 1024: classic-stat quarter
    HW2 = _N // 2

    with tile.TileContext(nc) as tc:
        with (
            tc.tile_pool(name="singles", bufs=1) as singles,
            tc.tile_pool(name="persist", bufs=1) as persist,
        ):
            # ---- input DMAs first on both trigger queues -----------------
            # x (fp8) chunks as SEPARATE half-tiles: DMA-written tiles are
            # waited at tile granularity, so a single whole-chunk tile would
            # stall the first stats op until both halves' queues drained.
            dma_engs = [nc.sync, nc.gpsimd]
            xth = []
            k = 0
            for ci in range(_CCH):
                halves = []
                for h in range(2):
                    xt = persist.tile([128, HW2], fp8, tag=f"xt{ci}_{h}",
                                      name=f"xt{ci}_{h}")
                    eng = dma_engs[k % 2]
                    k += 1
                    sl = slice(h * HW2, (h + 1) * HW2)
                    eng.dma_start(out=xt,
                                  in_=xb8[ci * 128:(ci + 1) * 128, sl])
                    halves.append(xt)
                xth.append(halves)

            # ---- constants / weights -------------------------------------
            sb_consts = singles.tile([128, 28], f32, tag="consts")
            nc.sync.dma_start(out=sb_consts, in_=consts[:, :])
            sb_bPa = sb_consts[:, 0:4]    # S_A*scale*(Wk^T bq)   (DVE copies)
            sb_bPb = sb_consts[:, 4:8]    # S_P*scale*(Wk^T bq)   (ACT copies)
            sb_bpe = sb_consts[:, 8:12]   # Wp bv + bp
            sb_gnw = sb_consts[:, 12:16]
            sb_gnb = sb_consts[:, 16:20]
            sb_gmat = sb_consts[:, 20:28]  # eye-repeat(8)/16
            sb_gexp = singles.tile([8, 128], f32, tag="gexp")
            nc.sync.dma_start(out=sb_gexp, in_=gexp[:, :])
            sb_obc16 = singles.tile([128, 128], bf16, tag="obc16")
            nc.sync.dma_start(out=sb_obc16, in_=obc16[:, :])
            sb_ones8 = singles.tile([128, 2, 128], fp8, tag="ones8")
            nc.sync.dma_start(
                out=sb_ones8, in_=obc8.rearrange("p (a f) -> p a f", a=2))
            # the two precombined weight matrices (needed ~30us in)
            w_tiles = {}
            for nm, src, eng in (("wP", wPT, nc.sync),
                                 ("wV", wVT, nc.gpsimd)):
                t = singles.tile([128, _CCH, _C], fp8, tag=f"w_{nm}",
                                 name=f"w_{nm}")
                eng.dma_start(
                    out=t, in_=src.rearrange("p (a f) -> p a f", a=_CCH))
                w_tiles[nm] = t

            sb_eps8 = singles.tile([8, 1], f32, tag="eps8")
            nc.vector.memset(sb_eps8, _EPS)
            sb_warm = singles.tile([128, 1], f32, tag="warm1")
            nc.vector.memset(sb_warm, 1.0)
            # ACT table prep: GN needs Sqrt/Square/Identity, all served by
            # the sqrt_and_others table.  Exp is touched once after the last
            # GN ACT op so the exp table load lands during the projections.
            sb_actw = singles.tile([8, 2], f32, tag="actw")
            nc.scalar.activation(out=sb_actw[:, 0:1], in_=sb_eps8, func=AF.Sqrt)
            nc.scalar.activation(out=sb_actw[:, 1:2], in_=sb_eps8,
                                 func=AF.Square)

            # normalize constants per chunk: mu', rstd', -mu'*rstd'
            musig = singles.tile([128, _CCH, 3], f32, tag="musig")

            # hn (normalized x, fp8) packed [c_lo, chunk, N]
            hn_t = persist.tile([128, _CCH, _N], fp8, tag="hn")

            # ---- phase 1: GroupNorm --------------------------------------
            with (
                tc.tile_pool(name="gn_small", bufs=2) as gn_small,
                tc.tile_pool(name="gn_psum", bufs=2, space="PSUM") as gn_psum,
                tc.tile_pool(name="warm_psum", bufs=1, space="PSUM") as warm_psum,
            ):
                # PE warm-up: keep the tensor engine busy through the GN head
                # so the HAM clock is ramped when the projections start.
                warm_ps = warm_psum.tile([128, 512], f32, tag="warm")

                def warm(n_small, n_big):
                    for _ in range(n_small):
                        nc.tensor.matmul(warm_ps[0:1, 0:1], lhsT=sb_warm,
                                         rhs=sb_warm, start=True, stop=True)
                    for _ in range(n_big):
                        nc.tensor.matmul(warm_ps, lhsT=xth[0][0][:, 0:128],
                                         rhs=xth[0][0][:, 0:512],
                                         start=True, stop=True)

                warm(80, 12)

                # ALL per-chunk stats first, then ONE batched chain for all
                # four chunks: the chain hops engines (PE/ACT round trips),
                # so running it per-chunk costs ~11us of sem latency while
                # batched it is ~4.  Chunks 0-2: single-pass BN_STATS on DVE
                # (earliest-arriving data); chunk 3: classic sum/sumsq on the
                # otherwise-idle ACT.
                t2all = gn_small.tile([128, _CCH, 2], f32, tag="t2all")
                sp3 = gn_small.tile([128, 4], f32, tag="sp3")
                qp3 = gn_small.tile([128, 4], f32, tag="qp3")
                for h in range(4):
                    src = xth[3][h // 2][:, (h % 2) * QW:(h % 2 + 1) * QW]
                    qs = slice(h * QW, (h + 1) * QW)
                    nc.scalar.activation(out=hn_t[:, 3, qs], in_=src,
                                         func=AF.Square,
                                         accum_out=qp3[:, h:h + 1])
                    nc.scalar.activation(out=hn_t[:, 3, qs], in_=src,
                                         func=AF.Identity,
                                         accum_out=sp3[:, h:h + 1])
                warm(0, 10)
                for ci in (0, 1, 2):
                    bn8 = gn_small.tile([128, 8, 6], f32, tag=f"bn8_{ci}",
                                        name=f"bn8_{ci}")
                    for s in range(8):
                        nc.vector.bn_stats(
                            out=bn8[:, s, :],
                            in_=xth[ci][s // 4][:, (s % 4) * 512:
                                                (s % 4 + 1) * 512])
                    mv = gn_small.tile([128, 2], f32, tag=f"mv_{ci}",
                                       name=f"mv_{ci}")
                    nc.vector.bn_aggr(out=mv, in_=bn8)
                    sqm = gn_small.tile([128, 1], f32, tag="sqm")
                    nc.vector.tensor_mul(sqm, mv[:, 0:1], mv[:, 0:1])
                    nc.vector.tensor_copy(out=t2all[:, ci, 0:1],
                                          in_=mv[:, 0:1])
                    nc.vector.tensor_tensor(
                        out=t2all[:, ci, 1:2], in0=mv[:, 1:2], in1=sqm,
                        op=OP.add)
                    warm(0, 7)
                nc.vector.reduce_sum(out=t2all[:, 3, 0:1], in_=sp3,
                                     axis=mybir.AxisListType.XYZW)
                nc.vector.reduce_sum(out=t2all[:, 3, 1:2], in_=qp3,
                                     axis=mybir.AxisListType.XYZW)
                nc.vector.tensor_scalar_mul(
                    out=t2all[:, 3, :], in0=t2all[:, 3, :], scalar1=1.0 / _N)

                # batched group chain for all four chunks (PSUM copies on
                # DVE so the ACT queue stays free)
                pg = gn_psum.tile([8, _CCH, 2], f32, tag="pg")
                nc.tensor.matmul(pg, lhsT=sb_gmat, rhs=t2all,
                                 start=True, stop=True)
                gs = gn_small.tile([8, _CCH, 2], f32, tag="gs")
                nc.vector.tensor_copy(out=gs, in_=pg)
                # var_g = m2 - mu^2 ; rstd_g = 1/sqrt(var+eps)
                musq = gn_small.tile([8, _CCH], f32, tag="musq")
                nc.vector.tensor_mul(musq, gs[:, :, 0], gs[:, :, 0])
                nc.vector.tensor_tensor(
                    out=gs[:, :, 1], in0=gs[:, :, 1], in1=musq,
                    op=OP.subtract)
                sq8 = gn_small.tile([8, _CCH], f32, tag="sq8")
                nc.scalar.activation(
                    out=sq8, in_=gs[:, :, 1], func=AF.Sqrt, bias=sb_eps8)
                nc.vector.reciprocal(out=gs[:, :, 1], in_=sq8)
                # broadcast to channels: gexp.T @ [mu_g, rstd_g]
                pc = gn_psum.tile([128, _CCH, 2], f32, tag="pc")
                nc.tensor.matmul(pc, lhsT=sb_gexp, rhs=gs, start=True,
                                 stop=True)
                pcs = gn_small.tile([128, _CCH, 2], f32, tag="pcs")
                nc.vector.tensor_copy(out=pcs, in_=pc)
                # fold gamma/beta: rstd' = rstd*gamma; mu' = mu - beta/rstd'
                nc.vector.tensor_mul(
                    musig[:, :, 1], pcs[:, :, 1], sb_gnw)
                rec = gn_small.tile([128, _CCH], f32, tag="rec")
                nc.vector.reciprocal(out=rec, in_=musig[:, :, 1])
                bs = gn_small.tile([128, _CCH], f32, tag="bs")
                nc.vector.tensor_mul(bs, sb_gnb, rec)
                nc.vector.tensor_tensor(
                    out=musig[:, :, 0], in0=pcs[:, :, 0], in1=bs,
                    op=OP.subtract)
                nc.vector.scalar_tensor_tensor(
                    out=musig[:, :, 2], in0=musig[:, :, 0],
                    scalar=-1.0, in1=musig[:, :, 1],
                    op0=OP.mult, op1=OP.mult)
                warm(0, 6)
                # hn = (x - mu') * rstd' (fp8), 5 halves on DVE (1.2us each)
                # and 3 on ACT (1.9us each) so both engines finish together
                for ci in range(_CCH):
                    for h in range(2):
                        hsl = slice(h * HW2, (h + 1) * HW2)
                        if ci < 2 or (ci == 2 and h == 0):
                            nc.vector.tensor_scalar(
                                out=hn_t[:, ci, hsl], in0=xth[ci][h],
                                scalar1=musig[:, ci, 0:1],
                                scalar2=musig[:, ci, 1:2],
                                op0=OP.subtract, op1=OP.mult)
                        else:
                            nc.scalar.activation(
                                out=hn_t[:, ci, hsl], in_=xth[ci][h],
                                func=AF.Identity, scale=musig[:, ci, 1:2],
                                bias=musig[:, ci, 2:3])
                warm(0, 4)
                # preload the exp table while the projections run
                nc.scalar.activation(out=sb_actw[:, 0:1], in_=sb_eps8,
                                     func=AF.Exp)
                warm(0, 8)

            # ---- phase 2: P and V'^T projections -------------------------
            p_t = persist.tile([128, _CCH, _NQ], fp8, tag="P")
            vt_t = persist.tile([128, 32, _C], fp8, tag="VT")

            with (
                tc.tile_pool(name="p2_psum", bufs=2, space="PSUM") as p2_psum,
            ):
                kq_psum = vt_psum = p2_psum
                # P[o]: queries only; copies alternate DVE/ACT
                for o in range(_CCH):
                    osl = slice(o * 128, (o + 1) * 128)
                    for jg in range(_NQ // 1024):
                        ps = kq_psum.tile([128, 2, 512], f32, tag="kq")
                        for jj in range(2):
                            j0 = jg * 1024 + jj * 512
                            for p in range(_CCH // 2):
                                nc.tensor.matmul(
                                    ps[:, jj, :],
                                    lhsT=w_tiles["wP"][:, 2 * p:2 * p + 2, osl],
                                    rhs=hn_t[:, 2 * p:2 * p + 2, j0:j0 + 512],
                                    start=(p == 0), stop=(p == _CCH // 2 - 1),
                                    perf_mode=DR)
                        dst = p_t[:, o, jg * 1024:(jg + 1) * 1024]
                        src = ps.rearrange("p a b -> p (a b)")
                        if jg % 2 == 0:
                            nc.vector.tensor_scalar(
                                out=dst, in0=src,
                                scalar1=sb_bPa[:, o:o + 1], scalar2=_SP / _SA,
                                op0=OP.add, op1=OP.mult)
                        else:
                            nc.scalar.activation(
                                out=dst, in_=src, func=AF.Identity,
                                scale=_SP / _SA, bias=sb_bPb[:, o:o + 1])
                # V'^T[j, c]: stationary = hn column slices; two j-tiles per
                # PSUM tile so each evacuation copy moves 1024 columns (the
                # copies, not the matmuls, are the V' throughput limit)
                for jp in range(16):
                    ps2 = vt_psum.tile([128, 2, 512], f32, tag="vt")
                    for jj in range(2):
                        jc = 2 * jp + jj
                        for p in range(_CCH // 2):
                            nc.tensor.matmul(
                                ps2[:, jj, :],
                                lhsT=hn_t[:, 2 * p:2 * p + 2,
                                          jc * 128:(jc + 1) * 128],
                                rhs=w_tiles["wV"][:, 2 * p:2 * p + 2, :],
                                start=(p == 0), stop=(p == _CCH // 2 - 1),
                                perf_mode=DR)
                    dst = vt_t[:, 2 * jp:2 * jp + 2, :].rearrange(
                        "p a b -> p (a b)")
                    src = ps2.rearrange("p a b -> p (a b)")
                    if jp % 2 == 0:
                        nc.scalar.mul(out=dst, in_=src, mul=_SV2 / _SV)
                    else:
                        nc.vector.tensor_scalar_mul(
                            out=dst, in0=src, scalar1=_SV2 / _SV)

            # ---- phase 3: attention + residual, per 512-query block ------
            with (
                tc.tile_pool(name="attw", bufs=1) as attw,
                tc.tile_pool(name="p3_psum", bufs=2, space="PSUM") as p3_psum,
            ):
                resw = attw     # merged pools (fewer close-drain rounds)
                s_psum = o_psum = pd_psum = p3_psum
                def attn_norm(osb, rbc_sb, cc, isl, on_pool):
                    """normalize by 1/denom, + (Wp bv + bp) + residual, out.
                    Half the chains ride the otherwise-idle Pool engine; the
                    rest use DVE's fused scalar_tensor_tensor (also: all of
                    the last block's, since Pool ops run ~1.3us each and
                    would stretch the kernel's tail)."""
                    xres = resw.tile([128, 512], f32, tag="xres", bufs=4)
                    nc.sync.dma_start(
                        out=xres, in_=xqf[cc * 128:(cc + 1) * 128, isl])
                    outt = resw.tile([128, 512], f32, tag="outt", bufs=4)
                    if on_pool:
                        t1 = resw.tile([128, 512], f32, tag="t1", bufs=4)
                        nc.gpsimd.tensor_tensor(
                            out=t1, in0=osb, in1=rbc_sb, op=OP.mult)
                        o1 = resw.tile([128, 512], f32, tag="o1", bufs=4)
                        nc.gpsimd.tensor_scalar(
                            out=o1, in0=t1, scalar1=sb_bpe[:, cc:cc + 1],
                            scalar2=1.0, op0=OP.add, op1=OP.mult)
                        nc.gpsimd.tensor_tensor(
                            out=outt, in0=o1, in1=xres, op=OP.add)
                        nc.gpsimd.dma_start(
                            out=out_d[cc * 128:(cc + 1) * 128, isl], in_=outt)
                    else:
                        t1 = resw.tile([128, 512], f32, tag="t1", bufs=4)
                        nc.vector.tensor_tensor(
                            out=t1, in0=osb, in1=rbc_sb, op=OP.mult)
                        nc.vector.scalar_tensor_tensor(
                            out=outt, in0=t1, scalar=sb_bpe[:, cc:cc + 1],
                            in1=xres, op0=OP.add, op1=OP.add)
                        nc.sync.dma_start(
                            out=out_d[cc * 128:(cc + 1) * 128, isl], in_=outt)

                for ib in range(_NQ // 512):
                    last = ib == (_NQ // 512) - 1
                    isl = slice(ib * 512, (ib + 1) * 512)
                    es = attw.tile([128, 32, 512], fp8, tag="ES", bufs=2)
                    l1 = attw.tile([128, 16, 512], bf16, tag="L1")
                    l2 = attw.tile([128, 2, 4, 512], bf16, tag="L2")
                    l3 = attw.tile([128, 2, 2, 512], bf16, tag="L3")
                    dhalf = attw.tile([128, 2, 512], bf16, tag="dh")
                    rbc = pd_psum.tile([128, 512], f32, tag="pd")
                    # The S^T matmuls alone outrun the exps (PSUM allows only
                    # 2 tiles in flight, so the PE would stall at the exps'
                    # pace); interleaving the AV accumulation for channel
                    # chunks 0/1 (whose es inputs are 2 groups behind) keeps
                    # the PE fed at exactly the rate ACT can sustain.
                    # Denominator add-tree in two halves (contiguous reads);
                    # the ones(x16)-matmul accumulates both halves into one
                    # PSUM bank and broadcasts the j-total to all partitions.
                    av01 = [o_psum.tile([128, 512], f32, tag="o",
                                        name=f"av{ib}_{c}")
                            for c in range(2)]

                    def av_mm(cc, jp, pso):
                        nc.tensor.matmul(
                            pso,
                            lhsT=vt_t[:, 2 * jp:2 * jp + 2,
                                      cc * 128:(cc + 1) * 128],
                            rhs=es[:, 2 * jp:2 * jp + 2, :],
                            start=(jp == 0), stop=(jp == 15),
                            perf_mode=DR)

                    for jg in range(16):
                        ps = s_psum.tile([128, 2, 512], f32, tag="s")
                        for jj in range(2):
                            jc = jg * 2 + jj
                            for p in range(_CCH // 2):
                                nc.tensor.matmul(
                                    ps[:, jj, :],
                                    lhsT=hn_t[:, 2 * p:2 * p + 2,
                                              jc * 128:(jc + 1) * 128],
                                    rhs=p_t[:, 2 * p:2 * p + 2, isl],
                                    start=(p == 0), stop=(p == _CCH // 2 - 1),
                                    perf_mode=DR)
                        if jg >= 2:
                            av_mm(0, jg - 2, av01[0])
                            av_mm(1, jg - 2, av01[1])
                            if jg >= 10:
                                # second denominator half summed by the PE
                                # (ones(x16) fp8 matmuls riding the es
                                # stream): 1.7us of PE buys back ~7us of the
                                # DVE add-tree, whose fp8 reads run at only
                                # ~1.3ns/col
                                nc.tensor.matmul(
                                    rbc, lhsT=sb_ones8,
                                    rhs=es[:, 2 * (jg - 2):2 * (jg - 1), :],
                                    start=(jg == 10), stop=False,
                                    perf_mode=DR)
                        nc.scalar.activation(
                            out=es[:, jg * 2:(jg + 1) * 2, :].rearrange(
                                "p a b -> p (a b)"),
                            in_=ps.rearrange("p a b -> p (a b)"),
                            func=AF.Exp, scale=1.0 / _SP)
                        if jg % 4 == 3 and jg < 8:
                            g = jg // 4
                            nc.vector.tensor_tensor(
                                out=l1[:, g * 4:(g + 1) * 4, :],
                                in0=es[:, 8 * g:8 * g + 4, :],
                                in1=es[:, 8 * g + 4:8 * (g + 1), :],
                                op=OP.add)
                        if jg == 7:
                            nc.vector.tensor_tensor(
                                out=l2[:, 0], in0=l1[:, 0:4, :],
                                in1=l1[:, 4:8, :], op=OP.add)
                            nc.vector.tensor_tensor(
                                out=l3[:, 0], in0=l2[:, 0, 0:2, :],
                                in1=l2[:, 0, 2:4, :], op=OP.add)
                            nc.vector.tensor_tensor(
                                out=dhalf[:, 0, :], in0=l3[:, 0, 0, :],
                                in1=l3[:, 0, 1, :], op=OP.add)
                    for jp in (14, 15):
                        av_mm(0, jp, av01[0])
                        av_mm(1, jp, av01[1])
                        nc.tensor.matmul(
                            rbc, lhsT=sb_ones8,
                            rhs=es[:, 2 * jp:2 * jp + 2, :],
                            start=False, stop=False, perf_mode=DR)
                    # O^T[c, i] = sum_j V'^T[j,c] expS^T[j,i]: this IS the
                    # (unnormalized) output -- no proj matmul afterwards.
                    # PSUM is evacuated by plain ACT copies so it never waits
                    # on the denominator; Pool/DVE finish the normalize.
                    # The rbc#2 matmul sits after AV(cc2) so the PE never
                    # waits for the trailing DVE half-tree.
                    rbc_sb = attw.tile([128, 512], f32, tag="rbc")
                    osbs = {}
                    for cc in (0, 1):
                        osb = resw.tile([128, 512], f32, tag="osb", bufs=4,
                                        name=f"osb{ib}_{cc}")
                        nc.scalar.copy(out=osb, in_=av01[cc])
                        osbs[cc] = osb
                    nc.tensor.matmul(rbc, lhsT=sb_obc16, rhs=dhalf[:, 0, :],
                                     start=False, stop=True)
                    nc.vector.reciprocal(out=rbc_sb, in_=rbc)
                    pso2 = o_psum.tile([128, 512], f32, tag="o")
                    for jp in range(16):
                        av_mm(2, jp, pso2)
                    attn_norm(osbs[0], rbc_sb, 0, isl, on_pool=not last)
                    attn_norm(osbs[1], rbc_sb, 1, isl, on_pool=False)
                    osb2 = resw.tile([128, 512], f32, tag="osb", bufs=4)
                    nc.scalar.copy(out=osb2, in_=pso2)
                    pso3 = o_psum.tile([128, 512], f32, tag="o")
                    for jp in range(16):
                        av_mm(3, jp, pso3)
                    attn_norm(osb2, rbc_sb, 2, isl, on_pool=not last)
                    osb3 = resw.tile([128, 512], f32, tag="osb", bufs=4)
                    nc.scalar.copy(out=osb3, in_=pso3)
                    attn_norm(osb3, rbc_sb, 3, isl, on_pool=False)

    _legalize_single_wait(nc, mybir)
    return nc


def kernel(**inputs):
    import ml_dtypes
    from concourse.bass_utils import run_bass_kernel_spmd

    global _cached
    if _cached is None:
        _cached = _build_program()
    nc = _cached

    x = np.asarray(inputs["x"], dtype=np.float32)
    gn_w = np.asarray(inputs["gn_w"], dtype=np.float32)
    gn_b = np.asarray(inputs["gn_b"], dtype=np.float32)
    wq = np.asarray(inputs["wq"], dtype=np.float32)
    bq = np.asarray(inputs["bq"], dtype=np.float32)
    wk = np.asarray(inputs["wk"], dtype=np.float32)
    wv = np.asarray(inputs["wv"], dtype=np.float32)
    bv = np.asarray(inputs["bv"], dtype=np.float32)
    wp = np.asarray(inputs["wp"], dtype=np.float32)
    bp = np.asarray(inputs["bp"], dtype=np.float32)
    # bk cancels in the j-softmax (it only adds per-query constants)

    fp8 = ml_dtypes.float8_e4m3
    scale = float(_C) ** -0.5

    def cols(v):  # [512] -> [128, 4] chunk columns
        return np.ascontiguousarray(v.reshape(_CCH, 128).T)

    def wlay(w, s):  # [cout, cin] -> lhsT chunked [128, cch*cout], fp8 x s
        return np.ascontiguousarray(
            w.T.reshape(_CCH, 128, _C).transpose(1, 0, 2).reshape(128, _CCH * _C)
            * s
        ).astype(fp8)

    wP = (wk.T.astype(np.float64) @ wq.astype(np.float64)).astype(np.float32)
    wV = (wp.astype(np.float64) @ wv.astype(np.float64)).astype(np.float32)
    cvec = wk.T @ bq

    consts = np.concatenate([
        cols(cvec * (_SA * scale)),                                 # bPa
        cols(cvec * (_SP * scale)),                                 # bPb
        cols(wp @ bv + bp),                                         # bpe2
        cols(gn_w),                                                 # gnw2
        cols(gn_b),                                                 # gnb2
        np.repeat(np.eye(8, dtype=np.float32), 16, axis=0) / 16.0,  # gmat
    ], axis=1)
    shared = {
        "wPT": wlay(wP, _SA * scale),
        "wVT": wlay(wV, _SV),
        "consts": consts,
        "obc16": np.full((128, 128), _SV2, ml_dtypes.bfloat16),
        "obc8": np.full((128, 256), _SV2, fp8),
        "gexp": np.repeat(np.eye(8, dtype=np.float32), 16, axis=1),
    }

    xf = x.reshape(_B, _C, _N)
    in_maps = []
    for core in range(_NCORES):
        bi, qh = core // 2, core % 2
        xbc = xf[bi]
        if qh == 1:  # rotate so this core's queries are columns 0..NQ-1
            xbc = np.concatenate([xbc[:, _NQ:], xbc[:, :_NQ]], axis=1)
        in_maps.append({
            "xb8": np.ascontiguousarray(xbc).astype(fp8),
            "xqf": np.ascontiguousarray(xbc[:, :_NQ], dtype=np.float32),
            **shared,
        })

    res = run_bass_kernel_spmd(nc, in_maps, core_ids=list(range(_NCORES)))

    out = np.empty((_B, _C, _N), np.float32)
    for core in range(_NCORES):
        bi, qh = core // 2, core % 2
        out[bi][:, qh * _NQ:(qh + 1) * _NQ] = res.results[core]["out"]
    return out.reshape(_B, _C, 64, 64)
